# revision 28
# baseline (speedup 1.0000x reference)
"""Fused conv3x3 -> GroupNorm(16) -> channel scale -> maxpool2x2 -> clamp[0,1]
Trainium2 Bass kernel, data-parallel over batch on 8 NeuronCores.

Input  x [32, 64, 128, 128] f32  -> output [32, 128, 63, 63] f32.
Each core handles 4 samples.

Conv: fp16 tap-pair matmuls, 5 PE passes per 8-row output group half
(vs 9 naive):
  - xt buffer: partition ci holds x[ci, row], ci+64 holds x[ci, row+1]
    -> one [128,128] stacked weight covers taps (kh0,kw)+(kh1,kw): 3 passes
  - xq buffer: both blocks hold x[ci, row+2], block1 shifted 1 column
    (loaded as flat row-major slices at +0/+1 element offsets, so both are
    single contiguous DMAs) -> taps (kh2,0)+(kh2,1) in 1 pass; (kh2,2)
    reads xq block0 at column offset 2 as a 64-contraction pass.
The two 64-contraction passes land on disjoint PE row groups so their
execution overlaps; PE busy ~147us/core is within ~15% of the MAC roofline.

Tail strategy (keeps the PE fed; DVE/ACT stay well under the PE's
~37us/sample):
  - GroupNorm stats as raw sums, no bn_stats: the ACT PSUM-evacuation Copy
    produces per-group sum(y) via accum_out; one DVE scalar_tensor_tensor
    per group (out=(y*1)*y -> scratch, accum_out=sum) produces sum(y^2) in
    a single 2x-mode pass.  (tensor_tensor_reduce compiles but dies at
    runtime on this toolchain; scalar_tensor_tensor's accum works.)
    Conv bias is folded analytically into the final per-channel affine.
  - the 8-channel group reduction is a tiny block-diagonal-ones matmul kept
    in fp16 (fp32r matmuls flush the PE pipeline on mode switch).
  - affine BEFORE pooling, as a single in-place DVE tensor_scalar over the
    whole sample -- contiguous fp16 SBUF operands hit the DVE 4x perf mode.
  - single max-pool branch, vertical pairs first (contiguous innermost ->
    DVE 2x mode), then horizontal pairs (strided, 1x), then fused clamp.
  - per-sample tails (coeff chain / affine / pools / store) are emitted
    interleaved with the next sample's conv chunks so no queue head-blocks.
  - output stores dispatch from the idle GpSimd SWDGE: on either hardware
    DGE queue (Sync carries x-loads, ACT carries PSUM evacs) the store's
    wait on the DVE clamp would head-block work the PE depends on.
fp16 output upcast to f32 on host.
"""

import numpy as np

import concourse.bacc as bacc
import concourse.mybir as mybir
import concourse.tile as tile
from concourse.bass_utils import run_bass_kernel_spmd

N_CORES = 8
B_FULL, CIN, H, W = 32, 64, 128, 128
COUT = 128
BPC = B_FULL // N_CORES  # samples per core
OH = OW = 126
PH = PW = 63
NG = 16  # groups
GSZ = COUT // NG  # 8 channels per group
EPS = 1e-5
S = OH * OW  # spatial size per sample
NEL = float(S)  # elements per channel for stats

# (x_row0, n_xrows, out_row0, n_out_rows)
CHUNKS = [(0, 10, 0, 8), (8, 10, 8, 8), (16, 10, 16, 8), (24, 14, 24, 12),
          (36, 18, 36, 16), (52, 24, 52, 22), (74, 24, 74, 22), (96, 24, 96, 22),
          (118, 10, 118, 8)]
XROWS_MAX = 24
NGROUPS = 17  # total 8-or-smaller row groups per sample
# groups whose sum(y^2) runs as an ACT Square pass instead of a DVE
# scalar_tensor_tensor pass -- both cost ~1.3us/group; a few go to ACT for
# balance, but their emission is deferred one chunk so the PSUM-evacuation
# copies always sit at the ACT queue head (evac latency gates the PE)
ACT_GROUPS = frozenset({2, 6, 10, 14})

_CACHED = {}


def _build():
    if "nc" in _CACHED:
        return _CACHED["nc"]
    f32 = mybir.dt.float32
    f16 = mybir.dt.float16
    AF = mybir.ActivationFunctionType
    OP = mybir.AluOpType

    nc = bacc.Bacc("TRN2", target_bir_lowering=False, debug=False)
    xs = nc.dram_tensor("xs", [BPC, CIN, H, W], f16, kind="ExternalInput").ap()
    wp_d = nc.dram_tensor("wp", [128, 3 * COUT], f16, kind="ExternalInput").ap()
    w2_d = nc.dram_tensor("w2", [128, COUT], f16, kind="ExternalInput").ap()
    ws_d = nc.dram_tensor("ws", [128, COUT], f16, kind="ExternalInput").ap()
    cb_d = nc.dram_tensor("cb", [COUT, 1], f32, kind="ExternalInput").ap()
    gs_d = nc.dram_tensor("gs", [COUT, 1], f32, kind="ExternalInput").ap()
    gbs_d = nc.dram_tensor("gbs", [COUT, 1], f32, kind="ExternalInput").ap()
    bones_d = nc.dram_tensor("bones", [COUT, COUT], f16, kind="ExternalInput").ap()
    out_d = nc.dram_tensor("out", [BPC, COUT, PH, PW], f16, kind="ExternalOutput").ap()

    with tile.TileContext(nc) as tc:
        with (
            tc.tile_pool(name="consts", bufs=1) as cpool,
            tc.tile_pool(name="xpool", bufs=5) as xpool,
            tc.tile_pool(name="xqpool", bufs=5) as xqpool,
            tc.tile_pool(name="ypool", bufs=2) as ypool,
            tc.tile_pool(name="sqpool", bufs=1) as sqpool,
            tc.tile_pool(name="stpool", bufs=2) as stpool,
            tc.tile_pool(name="pvpool", bufs=2) as pvpool,
            tc.tile_pool(name="popool", bufs=2) as popool,
            tc.tile_pool(name="cps", bufs=3, space="PSUM") as cps,
            tc.tile_pool(name="gps", bufs=1, space="PSUM") as gps,
        ):
            wp = cpool.tile([128, 3 * COUT], f16, name="wp_t")
            w2 = cpool.tile([128, COUT], f16, name="w2_t")
            ws = cpool.tile([128, COUT], f16, name="ws_t")
            cb = cpool.tile([COUT, 1], f32, name="cb_t")
            gs = cpool.tile([COUT, 1], f32, name="gs_t")
            gbs = cpool.tile([COUT, 1], f32, name="gbs_t")
            bones = cpool.tile([COUT, COUT], f16, name="bones_t")
            zeros1 = cpool.tile([COUT, 1], f32, name="zeros1")
            nc.vector.memset(zeros1[:], 0.0)
            # scratch for the y^2 main output (discarded; only accum is used)
            sqscr = sqpool.tile([128, 8 * OW], f16, name="sqscr")

            def load_chunk(b, xr0, nor):
                xt = xpool.tile([128, XROWS_MAX, W], f16, tag="x", name="xt")
                nc.sync.dma_start(
                    xt[0:64, 0:nor, :], xs[b, :, xr0 : xr0 + nor, :]
                )
                nc.sync.dma_start(
                    xt[64:128, 0:nor, :], xs[b, :, xr0 + 1 : xr0 + 1 + nor, :]
                )
                xq = xqpool.tile([128, XROWS_MAX, W], f16, tag="xq", name="xq")
                xf = xs[b].rearrange("c h w -> c (h w)")
                off = (xr0 + 2) * W
                nc.sync.dma_start(
                    xq[0:64, 0:nor, :].rearrange("p a b -> p (a b)"),
                    xf[:, off : off + nor * W],
                )
                n2 = min(nor * W, H * W - off - 1)
                nc.sync.dma_start(
                    xq[64:128, 0:nor, :].rearrange("p a b -> p (a b)")[:, 0:n2],
                    xf[:, off + 1 : off + 1 + n2],
                )
                return xt, xq

            # first matmul needs wp + chunk-0 x: issue those DMAs first, the
            # remaining consts (not needed until later matmuls / tails) after
            nc.sync.dma_start(wp[:], wp_d[:])
            prefetched = {(0, 0): load_chunk(0, CHUNKS[0][0], CHUNKS[0][3])}
            nc.sync.dma_start(w2[:], w2_d[:])
            nc.sync.dma_start(ws[:], ws_d[:])
            nc.sync.dma_start(cb[:], cb_d[:])
            nc.sync.dma_start(gs[:], gs_d[:])
            nc.sync.dma_start(gbs[:], gbs_d[:])
            nc.sync.dma_start(bones[:], bones_d[:])

            def tail_stats(tl):
                # full-sample st0 = E[y+cb], st1 = E[(y+cb)^2] from raw sums,
                # cast to fp16 for the group-reduction matmul
                S1 = stpool.tile([128, 1], f32, tag="S1", name="S1")
                nc.vector.tensor_reduce(S1[:], tl["s1c"][:],
                                        mybir.AxisListType.XYZW, OP.add)
                S2 = stpool.tile([128, 1], f32, tag="S2", name="S2")
                nc.vector.tensor_reduce(S2[:], tl["sqc"][:],
                                        mybir.AxisListType.XYZW, OP.add)
                st = stpool.tile([128, 2], f32, tag="sts", name="st")
                # st0 = S1/N + cb ; st1 = S2/N + cb*(2*S1/N + cb)
                nc.vector.tensor_scalar(st[:, 0:1], S1[:], 1.0 / NEL, cb[:],
                                        OP.mult, OP.add)
                t1 = stpool.tile([128, 1], f32, tag="t1", name="t1")
                nc.vector.tensor_scalar(t1[:], S1[:], 2.0 / NEL, cb[:],
                                        OP.mult, OP.add)
                nc.vector.tensor_tensor(t1[:], t1[:], cb[:], OP.mult)
                nc.vector.scalar_tensor_tensor(st[:, 1:2], S2[:], 1.0 / NEL,
                                               t1[:], OP.mult, OP.add)
                stf = stpool.tile([128, 2], f16, tag="stf", name="stf")
                nc.vector.tensor_scalar(stf[:], st[:], 1.0, None, OP.mult)
                tl["stv"] = stf

            def tail_gsum(tl):
                stf = tl["stv"]
                gsum = gps.tile([128, 2], f32, tag="gsum", name="gsum")
                nc.tensor.matmul(gsum[:], bones[:], stf[:], start=True, stop=True)
                mgrp = stpool.tile([128, 1], f32, tag="mgrp", name="mgrp")
                nc.vector.tensor_scalar(
                    mgrp[:], gsum[:, 0:1], 1.0 / GSZ, None, OP.mult
                )
                vgrp = stpool.tile([128, 1], f32, tag="vgrp", name="vgrp")
                nc.vector.tensor_scalar(
                    vgrp[:], gsum[:, 1:2], 1.0 / GSZ, EPS, OP.mult, OP.add
                )
                msq = stpool.tile([128, 1], f32, tag="msq", name="msq")
                nc.vector.tensor_tensor(msq[:], mgrp[:], mgrp[:], OP.mult)
                nc.vector.tensor_tensor(vgrp[:], vgrp[:], msq[:], OP.subtract)
                sdev = stpool.tile([128, 1], f32, tag="sdev", name="sdev")
                nc.scalar.activation(sdev[:], vgrp[:], AF.Sqrt, bias=zeros1[:])
                inv = stpool.tile([128, 1], f32, tag="inv", name="inv")
                nc.vector.reciprocal(inv[:], sdev[:])
                Acoef = stpool.tile([128, 1], f32, tag="Ac", name="Acoef")
                nc.vector.tensor_tensor(Acoef[:], inv[:], gs[:], OP.mult)
                # B = (cb - mgrp)*A + gbs
                Bcoef = stpool.tile([128, 1], f32, tag="Bc", name="Bcoef")
                nc.vector.scalar_tensor_tensor(Bcoef[:], cb[:], mgrp[:],
                                               Acoef[:], OP.subtract, OP.mult)
                nc.vector.tensor_tensor(Bcoef[:], Bcoef[:], gbs[:], OP.add)
                tl["A"], tl["B"] = Acoef, Bcoef

            def tail_affine(tl):
                # z = A*y + B in place, whole sample (DVE 4x mode)
                y = tl["y"]
                nc.vector.tensor_scalar(
                    y[:], y[:], tl["A"][:], tl["B"][:], OP.mult, OP.add
                )

            def tail_vpool(tl):
                # vertical max over row pairs: [126,126] -> [63,126]
                y3 = tl["y"][:].rearrange("p (a b) -> p a b", b=OW)
                pv = pvpool.tile([128, PH, OW], f16, tag="pv", name="pv")
                nc.vector.tensor_tensor(
                    pv[:], y3[:, 0:OH:2, :], y3[:, 1:OH:2, :], OP.max
                )
                tl["pv"] = pv

            def tail_hpool(tl, half):
                # horizontal max over col pairs + clamp: [63,126] -> [63,63],
                # split in two row-halves so each store can dispatch early
                pv = tl["pv"]
                if half == 0:
                    tl["po"] = popool.tile([128, PH, PW], f16, tag="po",
                                           name="po")
                po = tl["po"]
                r0, r1 = (0, 32) if half == 0 else (32, PH)
                nc.vector.tensor_tensor(
                    po[:, r0:r1, :], pv[:, r0:r1, 0:OW:2],
                    pv[:, r0:r1, 1:OW:2], OP.max
                )
                nc.vector.tensor_scalar(po[:, r0:r1, :], po[:, r0:r1, :],
                                        1.0, 0.0, OP.min, OP.max)

            def tail_store(tl, half):
                r0, r1 = (0, 32) if half == 0 else (32, PH)
                nc.gpsimd.dma_start(
                    out_d[tl["b"], :, r0:r1, :].rearrange("c h w -> c (h w)"),
                    tl["po"][:, r0:r1, :].rearrange("p a b -> p (a b)"),
                )

            pending = None
            for b in range(BPC):
                y_raw = ypool.tile([128, S], f16, tag="y", name="y_raw")
                s1cols = stpool.tile([128, NGROUPS], f32, tag="s1c", name="s1cols")
                sqcols = stpool.tile([128, NGROUPS], f32, tag="sqc", name="sqcols")

                gi = 0  # group index within sample
                defer_sq = []
                for ci, (xr0, nxr, or0, nor) in enumerate(CHUNKS):
                    # xt block0 = x rows xr0.., block1 = x rows xr0+1..;
                    # only `nor` rows each are read (kh2 taps come from xq).
                    # xq holds x rows (xr0+2) duplicated with a 1-column shift
                    # between partition blocks -> covers taps (kh2,kw0)+(kh2,kw1)
                    # in one 128-contraction matmul; block0 also serves (kh2,kw2)
                    to_emit = defer_sq
                    defer_sq = []
                    if (b, ci) in prefetched:
                        xt, xq = prefetched.pop((b, ci))
                    else:
                        xt, xq = load_chunk(b, xr0, nor)

                    g0 = or0
                    while g0 < or0 + nor:
                        gn = min(8, or0 + nor - g0)  # 8, 6 or 4 output rows
                        hr = gn // 2  # rows per half
                        cp = cps.tile([128, 1024], f32, tag="cp", name="cp")
                        for half in range(2):
                            row0 = g0 + half * hr
                            l0 = row0 - xr0
                            outap = cp[:, half * 512 : half * 512 + hr * OW]
                            for kw in range(3):
                                nc.tensor.matmul(
                                    outap,
                                    wp[:, kw * COUT : (kw + 1) * COUT],
                                    xt[:, l0 : l0 + hr, kw : kw + OW],
                                    start=(kw == 0),
                                    stop=False,
                                )
                            nc.tensor.matmul(
                                outap,
                                w2[:],
                                xq[:, l0 : l0 + hr, 0:OW],
                                start=False,
                                stop=False,
                            )
                        # (kh2,kw2) singles for both halves, adjacent on
                        # disjoint PE row groups (0-63 / 64-127) so the
                        # 16x 32x32 sub-arrays overlap their execution.
                        # half1 reads xq block1 (data shifted +1 col) at
                        # offset 1 -> x column c+2, same tap.
                        l0a = g0 - xr0
                        l0b = g0 + hr - xr0
                        nc.tensor.matmul(
                            cp[:, 0 : hr * OW],
                            ws[0:64, :],
                            xq[0:64, l0a : l0a + hr, 2 : 2 + OW],
                            start=False,
                            stop=True,
                            skip_group_check=True,
                        )
                        nc.tensor.matmul(
                            cp[:, 512 : 512 + hr * OW],
                            ws[64:128, :],
                            xq[64:128, l0b : l0b + hr, 1 : 1 + OW],
                            start=False,
                            stop=True,
                            skip_group_check=True,
                        )
                        # evacuate both halves in one strided ACT copy;
                        # accum_out gives this group's per-channel sum(y)
                        yv = y_raw[:, g0 * OW : (g0 + gn) * OW].rearrange(
                            "p (a b) -> p a b", b=hr * OW
                        )
                        nc.scalar.activation(
                            yv,
                            cp[:].rearrange("p (a b) -> p a b", b=512)[
                                :, :, 0 : hr * OW
                            ],
                            AF.Copy,
                            accum_out=s1cols[:, gi : gi + 1],
                        )
                        # sum(y^2): one pass writing y^2 to scratch with an
                        # add-reduce accumulator
                        yseg = y_raw[:, g0 * OW : (g0 + gn) * OW]
                        if gi in ACT_GROUPS:
                            defer_sq.append((yseg, gn, gi))
                        else:
                            nc.vector.scalar_tensor_tensor(
                                sqscr[:, 0 : gn * OW], yseg, 1.0, yseg,
                                OP.mult, OP.mult,
                                accum_out=sqcols[:, gi : gi + 1],
                            )
                        gi += 1
                        g0 += gn

                    # deferred ACT squares from previous chunks: emitted
                    # after this chunk's evacs so evacs lead the ACT queue
                    for yseg_d, gn_d, gi_d in to_emit:
                        nc.scalar.activation(
                            sqscr[:, 0 : gn_d * OW], yseg_d, AF.Square,
                            accum_out=sqcols[:, gi_d : gi_d + 1],
                        )

                    if b + 1 < BPC and ci in (5, 6):
                        nci = ci - 5
                        prefetched[(b + 1, nci)] = load_chunk(
                            b + 1, CHUNKS[nci][0], CHUNKS[nci][3]
                        )
                    if pending is not None:
                        if ci == 0:
                            tail_stats(pending)
                        elif ci == 1:
                            tail_gsum(pending)
                        elif ci == 2:
                            tail_affine(pending)
                        elif ci == 3:
                            tail_vpool(pending)
                        elif ci == 4:
                            tail_hpool(pending, 0)
                        elif ci == 5:
                            tail_store(pending, 0)
                            tail_hpool(pending, 1)
                        elif ci == 6:
                            tail_store(pending, 1)
                            pending = None

                for yseg_d, gn_d, gi_d in defer_sq:
                    nc.scalar.activation(
                        sqscr[:, 0 : gn_d * OW], yseg_d, AF.Square,
                        accum_out=sqcols[:, gi_d : gi_d + 1],
                    )
                pending = {"b": b, "s1c": s1cols, "sqc": sqcols, "y": y_raw}
            tail_stats(pending)
            tail_gsum(pending)
            tail_affine(pending)
            tail_vpool(pending)
            tail_hpool(pending, 0)
            tail_store(pending, 0)
            tail_hpool(pending, 1)
            tail_store(pending, 1)
    nc.finalize()
    _CACHED["nc"] = nc
    return nc


def _prep_consts(conv_w, conv_b, gn_w, gn_b, scale):
    # wp[ci + 64*kh, kw*COUT + co] = conv_w[co, ci, kh, kw] for kh in {0,1}
    # w2[ci, co] = conv_w[co, ci, 2, 0]; w2[64+ci, co] = conv_w[co, ci, 2, 1]
    # ws[ci, co] = conv_w[co, ci, 2, 2]
    w = np.ascontiguousarray(conv_w.astype(np.float32))
    wp = np.empty((128, 3 * COUT), np.float16)
    w2 = np.empty((128, COUT), np.float16)
    ws = np.empty((128, COUT), np.float16)
    for kw in range(3):
        wp[0:64, kw * COUT : (kw + 1) * COUT] = w[:, :, 0, kw].T
        wp[64:128, kw * COUT : (kw + 1) * COUT] = w[:, :, 1, kw].T
    w2[0:64, :] = w[:, :, 2, 0].T
    w2[64:128, :] = w[:, :, 2, 1].T
    ws[0:64, :] = w[:, :, 2, 2].T
    ws[64:128, :] = w[:, :, 2, 2].T
    cb = conv_b.astype(np.float32).reshape(COUT, 1)
    sc = scale.astype(np.float32).reshape(COUT)
    gs = (gn_w.astype(np.float32) * sc).reshape(COUT, 1)
    gbs = (gn_b.astype(np.float32) * sc).reshape(COUT, 1)
    bones = np.zeros((COUT, COUT), np.float16)
    for g in range(NG):
        bones[g * GSZ : (g + 1) * GSZ, g * GSZ : (g + 1) * GSZ] = 1.0
    return wp, w2, ws, cb, gs, gbs, bones


def kernel(x, conv_w, conv_b, gn_w, gn_b, scale):
    x = np.asarray(x, dtype=np.float32).astype(np.float16)
    wp, w2, ws, cb, gs, gbs, bones = _prep_consts(
        np.asarray(conv_w), np.asarray(conv_b), np.asarray(gn_w),
        np.asarray(gn_b), np.asarray(scale),
    )
    nc = _build()
    in_maps = []
    for c in range(N_CORES):
        in_maps.append({
            "xs": x[c * BPC : (c + 1) * BPC],
            "wp": wp, "w2": w2, "ws": ws,
            "cb": cb, "gs": gs, "gbs": gbs, "bones": bones,
        })
    results = _run_cached(nc, in_maps)
    out = np.concatenate([results[c]["out"] for c in range(N_CORES)], axis=0)
    return out.astype(np.float32)


def _run_cached(nc, in_maps):
    """run_bass_kernel_spmd's axon path with the jitted executable cached
    across calls (avoids re-tracing the shard_map wrapper every call)."""
    import jax
    import numpy as _np
    from jax.sharding import Mesh, PartitionSpec
    from jax.experimental.shard_map import shard_map
    from concourse import bass2jax

    if "runner" not in _CACHED:
        bass2jax.install_neuronx_cc_hook()
        partition_name = (
            nc.partition_id_tensor.name if nc.partition_id_tensor else None
        )
        in_names, out_names, out_avals, zero_outs = [], [], [], []
        for alloc in nc.m.functions[0].allocations:
            if not isinstance(alloc, mybir.MemoryLocationSet):
                continue
            name = alloc.memorylocations[0].name
            if alloc.kind == "ExternalInput":
                if name != partition_name:
                    in_names.append(name)
            elif alloc.kind == "ExternalOutput":
                shape = tuple(alloc.tensor_shape)
                dtype = mybir.dt.np(alloc.dtype)
                out_names.append(name)
                out_avals.append(jax.core.ShapedArray(shape, dtype))
                zero_outs.append(_np.zeros(shape, dtype))
        n_params = len(in_names)
        n_outs = len(out_avals)
        all_names = list(in_names) + list(out_names)
        if partition_name is not None:
            all_names.append(partition_name)
        donate = tuple(range(n_params, n_params + n_outs))

        def _body(*args):
            operands = list(args)
            if partition_name is not None:
                operands.append(bass2jax.partition_id_tensor())
            outs = bass2jax._bass_exec_p.bind(
                *operands,
                out_avals=tuple(out_avals),
                in_names=tuple(all_names),
                out_names=tuple(out_names),
                lowering_input_output_aliases=(),
                sim_require_finite=True,
                sim_require_nnan=True,
                nc=nc,
            )
            return tuple(outs)

        devices = jax.devices()[:N_CORES]
        mesh = Mesh(_np.asarray(devices), ("core",))
        in_specs = (PartitionSpec("core"),) * (n_params + n_outs)
        out_specs = (PartitionSpec("core"),) * n_outs
        sharded = jax.jit(
            shard_map(_body, mesh=mesh, in_specs=in_specs,
                      out_specs=out_specs, check_rep=False),
            donate_argnums=donate, keep_unused=True,
        )
        _CACHED["runner"] = (sharded, in_names, out_names, out_avals, zero_outs)

    sharded, in_names, out_names, out_avals, zero_outs = _CACHED["runner"]
    import numpy as _np2
    concat_in = [
        _np2.concatenate([_np2.asarray(in_maps[c][n]) for c in range(N_CORES)], axis=0)
        for n in in_names
    ]
    concat_zeros = [
        _np2.zeros((N_CORES * z.shape[0], *z.shape[1:]), z.dtype) for z in zero_outs
    ]
    out_arrs = sharded(*concat_in, *concat_zeros)
    return [
        {
            name: _np2.asarray(out_arrs[i]).reshape(N_CORES, *out_avals[i].shape)[c]
            for i, name in enumerate(out_names)
        }
        for c in range(N_CORES)
    ]


if __name__ == "__main__":
    rng = np.random.default_rng(0)
    x = rng.standard_normal((B_FULL, CIN, H, W), dtype=np.float32)
    cw = rng.standard_normal((COUT, CIN, 3, 3), dtype=np.float32)
    out = kernel(x, cw, rng.standard_normal(COUT, dtype=np.float32),
                 rng.standard_normal(COUT, dtype=np.float32),
                 rng.standard_normal(COUT, dtype=np.float32),
                 rng.standard_normal((COUT, 1, 1), dtype=np.float32))
    print(out.shape, out.dtype)


# revision 29
# speedup vs baseline: 1.0550x; 1.0550x over previous
"""Fused conv3x3 -> GroupNorm(16) -> channel scale -> maxpool2x2 -> clamp[0,1]
Trainium2 Bass kernel, data-parallel over batch on 8 NeuronCores.

Input  x [32, 64, 128, 128] f32  -> output [32, 128, 63, 63] f32.
Each core handles 4 samples.

Conv: fp16 tap-pair matmuls, 5 PE passes per 8-row output group half
(vs 9 naive):
  - xt buffer: partition ci holds x[ci, row], ci+64 holds x[ci, row+1]
    -> one [128,128] stacked weight covers taps (kh0,kw)+(kh1,kw): 3 passes
  - xq buffer: both blocks hold x[ci, row+2], block1 shifted 1 column
    (loaded as flat row-major slices at +0/+1 element offsets, so both are
    single contiguous DMAs) -> taps (kh2,0)+(kh2,1) in 1 pass; (kh2,2)
    reads xq block0 at column offset 2 as a 64-contraction pass.
The two 64-contraction passes land on disjoint PE row groups so their
execution overlaps; PE busy ~157us/core is within ~20% of the MAC roofline.

Tail strategy (keeps the PE fed; DVE and ACT each stay under the PE's
~39us/sample):
  - stats WITHOUT bn_stats: the ACT PSUM-evacuation Copy produces per-group
    sum(y) via accum_out; one DVE tensor_tensor_reduce per group (y*y ->
    scratch, accum add) produces sum(y^2). Conv bias is folded analytically.
  - affine BEFORE pooling, as a single in-place DVE tensor_scalar over the
    whole sample -- contiguous fp16 SBUF operands hit the DVE 4x perf mode
    (~0.26 ns/el), so this is 3x cheaper than splitting affine over ACT+DVE.
  - single max-pool branch, vertical pairs first (contiguous innermost ->
    DVE 2x mode), then horizontal pairs (strided, 1x), then fused clamp.
  - per-sample tails (coeff chain / affine / pools / store) are emitted
    interleaved with the next sample's conv chunks so no queue head-blocks.
fp16 output upcast to f32 on host.
"""

import numpy as np

import concourse.bacc as bacc
import concourse.mybir as mybir
import concourse.tile as tile
from concourse.bass_utils import run_bass_kernel_spmd

N_CORES = 8
B_FULL, CIN, H, W = 32, 64, 128, 128
COUT = 128
BPC = B_FULL // N_CORES  # samples per core
OH = OW = 126
PH = PW = 63
NG = 16  # groups
GSZ = COUT // NG  # 8 channels per group
EPS = 1e-5
S = OH * OW  # spatial size per sample
NEL = float(S)  # elements per channel for stats

# (x_row0, n_xrows, out_row0, n_out_rows)
CHUNKS = [(0, 10, 0, 8), (8, 10, 8, 8), (16, 10, 16, 8), (24, 14, 24, 12),
          (36, 18, 36, 16), (52, 24, 52, 22), (74, 24, 74, 22), (96, 24, 96, 22),
          (118, 10, 118, 8)]
XROWS_MAX = 24
NGROUPS = 17  # total 8-or-smaller row groups per sample
# row count per group (CHUNKS split into <=8-row groups)
GROUP_ROWS = [8, 8, 8, 8, 4, 8, 8, 8, 8, 6, 8, 8, 6, 8, 8, 6, 8]
# groups whose stats run on ACT (Square+accum); rest use DVE bn_stats.
# the last sample shifts most groups to ACT: its DVE also carries the
# previous sample's tail plus the last-sample max+min pools, and any DVE
# backlog there directly lengthens the serial post-conv tail
A6 = frozenset({1, 4, 7, 10, 13, 16})
A12 = frozenset(range(NGROUPS)) - {0, 4, 8, 12, 16}
ACT_SETS = [A6, A6, A12, A12]
# samples 2,3 pool during their own conv (max+min branches) and get a short
# all-DVE affine-combine tail; samples 0,1 use the cheaper affine-first tail
POOL_FIRST = (False, False, True, True)
N_ACT_MAX = max(len(s) for s in ACT_SETS)
N_BN_MAX = max(NGROUPS - len(s) for s in ACT_SETS)

_CACHED = {}


def _build():
    if "nc" in _CACHED:
        return _CACHED["nc"]
    f32 = mybir.dt.float32
    f16 = mybir.dt.float16
    AF = mybir.ActivationFunctionType
    OP = mybir.AluOpType

    nc = bacc.Bacc("TRN2", target_bir_lowering=False, debug=False)
    xs = nc.dram_tensor("xs", [BPC, CIN, H, W], f16, kind="ExternalInput").ap()
    wp_d = nc.dram_tensor("wp", [128, 3 * COUT], f16, kind="ExternalInput").ap()
    w2_d = nc.dram_tensor("w2", [128, COUT], f16, kind="ExternalInput").ap()
    ws_d = nc.dram_tensor("ws", [128, COUT], f16, kind="ExternalInput").ap()
    cb_d = nc.dram_tensor("cb", [COUT, 1], f32, kind="ExternalInput").ap()
    gs_d = nc.dram_tensor("gs", [COUT, 1], f32, kind="ExternalInput").ap()
    gbs_d = nc.dram_tensor("gbs", [COUT, 1], f32, kind="ExternalInput").ap()
    bones_d = nc.dram_tensor("bones", [COUT, COUT], f32, kind="ExternalInput").ap()
    out_d = nc.dram_tensor("out", [BPC, COUT, PH, PW], f16, kind="ExternalOutput").ap()

    with tile.TileContext(nc) as tc:
        with (
            tc.tile_pool(name="consts", bufs=1) as cpool,
            tc.tile_pool(name="xpool", bufs=3) as xpool,
            tc.tile_pool(name="xqpool", bufs=3) as xqpool,
            tc.tile_pool(name="ypool", bufs=2) as ypool,
            tc.tile_pool(name="sqpool", bufs=1) as sqpool,
            tc.tile_pool(name="stpool", bufs=2) as stpool,
            tc.tile_pool(name="pvpool", bufs=2) as pvpool,
            tc.tile_pool(name="popool", bufs=2) as popool,
            tc.tile_pool(name="vspool", bufs=2) as vspool,
            tc.tile_pool(name="hxpool", bufs=2) as hxpool,
            tc.tile_pool(name="cps", bufs=3, space="PSUM") as cps,
            tc.tile_pool(name="gps", bufs=1, space="PSUM") as gps,
        ):
            wp = cpool.tile([128, 3 * COUT], f16, name="wp_t")
            w2 = cpool.tile([128, COUT], f16, name="w2_t")
            ws = cpool.tile([128, COUT], f16, name="ws_t")
            cb = cpool.tile([COUT, 1], f32, name="cb_t")
            gs = cpool.tile([COUT, 1], f32, name="gs_t")
            gbs = cpool.tile([COUT, 1], f32, name="gbs_t")
            bones = cpool.tile([COUT, COUT], f32, name="bones_t")
            zeros1 = cpool.tile([COUT, 1], f32, name="zeros1")
            nc.vector.memset(zeros1[:], 0.0)

            def load_chunk(b, xr0, nor):
                xt = xpool.tile([128, XROWS_MAX, W], f16, tag="x", name="xt")
                nc.sync.dma_start(
                    xt[0:64, 0:nor, :], xs[b, :, xr0 : xr0 + nor, :]
                )
                nc.sync.dma_start(
                    xt[64:128, 0:nor, :], xs[b, :, xr0 + 1 : xr0 + 1 + nor, :]
                )
                xq = xqpool.tile([128, XROWS_MAX, W], f16, tag="xq", name="xq")
                xf = xs[b].rearrange("c h w -> c (h w)")
                off = (xr0 + 2) * W
                nc.sync.dma_start(
                    xq[0:64, 0:nor, :].rearrange("p a b -> p (a b)"),
                    xf[:, off : off + nor * W],
                )
                n2 = min(nor * W, H * W - off - 1)
                nc.sync.dma_start(
                    xq[64:128, 0:nor, :].rearrange("p a b -> p (a b)")[:, 0:n2],
                    xf[:, off + 1 : off + 1 + n2],
                )
                return xt, xq

            # first matmul needs wp + chunk-0 x: issue those DMAs first, the
            # remaining consts (not needed until later matmuls / tails) after
            nc.sync.dma_start(wp[:], wp_d[:])
            prefetch = load_chunk(0, CHUNKS[0][0], CHUNKS[0][3])
            nc.sync.dma_start(w2[:], w2_d[:])
            nc.sync.dma_start(ws[:], ws_d[:])
            nc.sync.dma_start(cb[:], cb_d[:])
            nc.sync.dma_start(gs[:], gs_d[:])
            nc.sync.dma_start(gbs[:], gbs_d[:])
            nc.sync.dma_start(bones[:], bones_d[:])
            # scratch for the ACT Square main output (discarded)
            sqscr = sqpool.tile([128, 8 * OW], f16, name="sqscr")

            def tail_stats(tl):
                # merge bn_stats Welford aggregate (N_BN_ELS els) with the
                # ACT-square raw sums (N_ACT_ELS els) into full-sample
                # st0 = E[y+cb], st1 = E[(y+cb)^2]
                n_bn_els = NEL - tl["n_act_els"]
                mv = stpool.tile([128, 2], f32, tag="mv", name="mv")
                nc.vector.bn_aggr(mv[:], tl["st"][:, 0 : 2 * tl["n_bn"], :])
                S1 = stpool.tile([128, 1], f32, tag="S1", name="S1")
                nc.vector.tensor_reduce(S1[:], tl["s1c"][:, 0 : tl["n_act"]],
                                        mybir.AxisListType.XYZW, OP.add)
                S2 = stpool.tile([128, 1], f32, tag="S2", name="S2")
                nc.vector.tensor_reduce(S2[:], tl["sqc"][:, 0 : tl["n_act"]],
                                        mybir.AxisListType.XYZW, OP.add)
                # S1 <- S1 + mean_a * N_a ; S2 <- S2 + (var_a+mean_a^2) * N_a
                t0 = stpool.tile([128, 1], f32, tag="t0", name="t0")
                nc.vector.tensor_scalar(t0[:], mv[:, 0:1], n_bn_els, S1[:],
                                        OP.mult, OP.add)
                t2 = stpool.tile([128, 1], f32, tag="t2", name="t2")
                nc.vector.tensor_tensor(t2[:], mv[:, 0:1], mv[:, 0:1], OP.mult)
                nc.vector.tensor_tensor(t2[:], t2[:], mv[:, 1:2], OP.add)
                nc.vector.tensor_scalar(t2[:], t2[:], n_bn_els, S2[:],
                                        OP.mult, OP.add)
                st = stpool.tile([128, 2], f32, tag="sts", name="st")
                # st0 = S1/N + cb ; st1 = S2/N + cb*(2*S1/N + cb)
                nc.vector.tensor_scalar(st[:, 0:1], t0[:], 1.0 / NEL, cb[:],
                                        OP.mult, OP.add)
                t1 = stpool.tile([128, 1], f32, tag="t1", name="t1")
                nc.vector.tensor_scalar(t1[:], t0[:], 2.0 / NEL, cb[:],
                                        OP.mult, OP.add)
                nc.vector.tensor_tensor(t1[:], t1[:], cb[:], OP.mult)
                nc.vector.tensor_scalar(st[:, 1:2], t2[:], 1.0 / NEL, None,
                                        OP.mult)
                nc.vector.tensor_tensor(st[:, 1:2], st[:, 1:2], t1[:], OP.add)
                tl["stv"] = st

            def tail_gsum(tl):
                st = tl["stv"]
                gsum = gps.tile([128, 2], f32, tag="gsum", name="gsum")
                nc.tensor.matmul(gsum[:], bones[:], st[:], start=True, stop=True)
                mgrp = stpool.tile([128, 1], f32, tag="mgrp", name="mgrp")
                nc.vector.tensor_scalar(
                    mgrp[:], gsum[:, 0:1], 1.0 / GSZ, None, OP.mult
                )
                vgrp = stpool.tile([128, 1], f32, tag="vgrp", name="vgrp")
                nc.vector.tensor_scalar(
                    vgrp[:], gsum[:, 1:2], 1.0 / GSZ, EPS, OP.mult, OP.add
                )
                msq = stpool.tile([128, 1], f32, tag="msq", name="msq")
                nc.vector.tensor_tensor(msq[:], mgrp[:], mgrp[:], OP.mult)
                nc.vector.tensor_tensor(vgrp[:], vgrp[:], msq[:], OP.subtract)
                sdev = stpool.tile([128, 1], f32, tag="sdev", name="sdev")
                nc.scalar.activation(sdev[:], vgrp[:], AF.Sqrt, bias=zeros1[:])
                inv = stpool.tile([128, 1], f32, tag="inv", name="inv")
                nc.vector.reciprocal(inv[:], sdev[:])
                Acoef = stpool.tile([128, 1], f32, tag="Ac", name="Acoef")
                nc.vector.tensor_tensor(Acoef[:], inv[:], gs[:], OP.mult)
                Bcoef = stpool.tile([128, 1], f32, tag="Bc", name="Bcoef")
                nc.vector.tensor_tensor(Bcoef[:], cb[:], mgrp[:], OP.subtract)
                nc.vector.tensor_tensor(Bcoef[:], Bcoef[:], Acoef[:], OP.mult)
                nc.vector.tensor_tensor(Bcoef[:], Bcoef[:], gbs[:], OP.add)
                tl["A"], tl["B"] = Acoef, Bcoef

            def tail_affine(tl):
                # z = A*y + B in place, whole sample (DVE 4x mode)
                y = tl["y"]
                nc.vector.tensor_scalar(
                    y[:], y[:], tl["A"][:], tl["B"][:], OP.mult, OP.add
                )

            def tail_vpool(tl):
                # vertical max over row pairs: [126,126] -> [63,126]
                y3 = tl["y"][:].rearrange("p (a b) -> p a b", b=OW)
                pv = pvpool.tile([128, PH, OW], f16, tag="pv", name="pv")
                nc.vector.tensor_tensor(
                    pv[:], y3[:, 0:OH:2, :], y3[:, 1:OH:2, :], OP.max
                )
                tl["pv"] = pv

            def tail_hpool(tl):
                # horizontal max over col pairs + clamp: [63,126] -> [63,63]
                pv = tl["pv"]
                po = popool.tile([128, PH, PW], f16, tag="po", name="po")
                nc.vector.tensor_tensor(
                    po[:], pv[:, :, 0:OW:2], pv[:, :, 1:OW:2], OP.max
                )
                nc.vector.tensor_scalar(po[:], po[:], 1.0, 0.0, OP.min, OP.max)
                tl["po"] = po

            def tail_combine(tl):
                # z = clamp(max(A*maxpool(y)+B, A*minpool(y)+B), 0, 1)
                # == clamp(maxpool(A*y+B)) for either sign of A
                Acoef, Bcoef = tl["A"], tl["B"]
                hx_, hn_ = tl["hx"], tl["hn"]
                po = popool.tile([128, PH, PW], f16, tag="po", name="po")
                nc.vector.tensor_scalar(
                    po[:], hx_[:], Acoef[:], Bcoef[:], OP.mult, OP.add
                )
                nc.vector.tensor_scalar(
                    hn_[:], hn_[:], Acoef[:], Bcoef[:], OP.mult, OP.add
                )
                nc.vector.tensor_tensor(po[:], po[:], hn_[:], OP.max)
                nc.vector.tensor_scalar(po[:], po[:], 1.0, 0.0, OP.min, OP.max)
                tl["po"] = po

            def tail_store(tl):
                # dispatch from the idle GpSimd SWDGE: the store waits on the
                # DVE clamp, and on either hardware DGE queue (Sync carries
                # x-loads, ACT carries PSUM evacs) it would head-block work
                # the PE depends on, starving it for ~5-10us per sample
                nc.gpsimd.dma_start(
                    out_d[tl["b"]].rearrange("c h w -> c (h w)"),
                    tl["po"][:].rearrange("p a b -> p (a b)"),
                )

            pending = None
            for b in range(BPC):
                pool_first = POOL_FIRST[b]
                if pool_first:
                    hx = hxpool.tile([128, PH, PW], f16, tag="hx", name="hx")
                    hn = hxpool.tile([128, PH, PW], f16, tag="hn", name="hn")
                y_raw = ypool.tile([128, S], f16, tag="y", name="y_raw")
                act_set = ACT_SETS[b]
                n_act = len(act_set)
                n_bn = NGROUPS - n_act
                n_act_els = float(sum(GROUP_ROWS[g] for g in act_set) * OW)
                s1cols = stpool.tile([128, N_ACT_MAX], f32, tag="s1c", name="s1cols")
                sqcols = stpool.tile([128, N_ACT_MAX], f32, tag="sqc", name="sqcols")
                stats = stpool.tile([128, 2 * N_BN_MAX, 6], f32,
                                    tag="st", name="stats")

                gi = 0  # group index within sample
                aci = 0  # ACT-square group counter
                si = 0  # bn_stats slot counter
                for ci, (xr0, nxr, or0, nor) in enumerate(CHUNKS):
                    # xt block0 = x rows xr0.., block1 = x rows xr0+1..;
                    # only `nor` rows each are read (kh2 taps come from xq).
                    # xq holds x rows (xr0+2) duplicated with a 1-column shift
                    # between partition blocks -> covers taps (kh2,kw0)+(kh2,kw1)
                    # in one 128-contraction matmul; block0 also serves (kh2,kw2)
                    if b == 0 and ci == 0:
                        xt, xq = prefetch
                    else:
                        xt, xq = load_chunk(b, xr0, nor)

                    g0 = or0
                    while g0 < or0 + nor:
                        gn = min(8, or0 + nor - g0)  # 8, 6 or 4 output rows
                        hr = gn // 2  # rows per half
                        cp = cps.tile([128, 1024], f32, tag="cp", name="cp")
                        for half in range(2):
                            row0 = g0 + half * hr
                            l0 = row0 - xr0
                            outap = cp[:, half * 512 : half * 512 + hr * OW]
                            for kw in range(3):
                                nc.tensor.matmul(
                                    outap,
                                    wp[:, kw * COUT : (kw + 1) * COUT],
                                    xt[:, l0 : l0 + hr, kw : kw + OW],
                                    start=(kw == 0),
                                    stop=False,
                                )
                            nc.tensor.matmul(
                                outap,
                                w2[:],
                                xq[:, l0 : l0 + hr, 0:OW],
                                start=False,
                                stop=False,
                            )
                        # (kh2,kw2) singles for both halves, adjacent on
                        # disjoint PE row groups (0-63 / 64-127) so the
                        # 16x 32x32 sub-arrays overlap their execution.
                        # half1 reads xq block1 (data shifted +1 col) at
                        # offset 1 -> x column c+2, same tap.
                        l0a = g0 - xr0
                        l0b = g0 + hr - xr0
                        nc.tensor.matmul(
                            cp[:, 0 : hr * OW],
                            ws[0:64, :],
                            xq[0:64, l0a : l0a + hr, 2 : 2 + OW],
                            start=False,
                            stop=True,
                            skip_group_check=True,
                        )
                        nc.tensor.matmul(
                            cp[:, 512 : 512 + hr * OW],
                            ws[64:128, :],
                            xq[64:128, l0b : l0b + hr, 1 : 1 + OW],
                            start=False,
                            stop=True,
                            skip_group_check=True,
                        )
                        # evacuate both halves in one strided ACT copy;
                        # accum_out gives this group's per-channel sum(y)
                        yv = y_raw[:, g0 * OW : (g0 + gn) * OW].rearrange(
                            "p (a b) -> p a b", b=hr * OW
                        )
                        on_act = gi in act_set
                        nc.scalar.activation(
                            yv,
                            cp[:].rearrange("p (a b) -> p a b", b=512)[
                                :, :, 0 : hr * OW
                            ],
                            AF.Copy,
                            accum_out=(s1cols[:, aci : aci + 1]
                                       if on_act else None),
                        )
                        yseg = y_raw[:, g0 * OW : (g0 + gn) * OW]
                        if on_act:
                            # ACT square pass: accum gives sum(y^2)
                            nc.scalar.activation(
                                sqscr[:, 0 : gn * OW], yseg, AF.Square,
                                accum_out=sqcols[:, aci : aci + 1],
                            )
                            aci += 1
                        else:
                            # DVE one-pass Welford stats per 504-el half
                            for half in range(2):
                                r0 = (g0 + half * hr) * OW
                                nc.vector.bn_stats(
                                    stats[:, si, :],
                                    y_raw[:, r0 : r0 + hr * OW],
                                )
                                si += 1
                        gi += 1
                        g0 += gn

                    if pool_first:
                        # pool during the sample's own conv (max+min branches
                        # on raw y) so only a short affine-combine tail is left
                        y3c = y_raw[:].rearrange("p (a b) -> p a b", b=OW)
                        h0 = or0 // 2
                        nh = nor // 2
                        vx = vspool.tile([128, 11, OW], f16, tag="vx", name="vx")
                        vn = vspool.tile([128, 11, OW], f16, tag="vn", name="vn")
                        nc.vector.tensor_tensor(
                            vx[:, 0:nh, :],
                            y3c[:, or0 : or0 + nor : 2, :],
                            y3c[:, or0 + 1 : or0 + nor : 2, :],
                            OP.max,
                        )
                        nc.vector.tensor_tensor(
                            vn[:, 0:nh, :],
                            y3c[:, or0 : or0 + nor : 2, :],
                            y3c[:, or0 + 1 : or0 + nor : 2, :],
                            OP.min,
                        )
                        nc.vector.tensor_tensor(
                            hx[:, h0 : h0 + nh, :],
                            vx[:, 0:nh, 0 : OW : 2],
                            vx[:, 0:nh, 1 : OW : 2],
                            OP.max,
                        )
                        nc.vector.tensor_tensor(
                            hn[:, h0 : h0 + nh, :],
                            vn[:, 0:nh, 0 : OW : 2],
                            vn[:, 0:nh, 1 : OW : 2],
                            OP.min,
                        )
                    if pending is not None:
                        if ci == 0:
                            tail_stats(pending)
                        elif ci == 1:
                            tail_gsum(pending)
                        elif pending["pf"]:
                            if ci == 2:
                                tail_combine(pending)
                            elif ci == 3:
                                tail_store(pending)
                                pending = None
                        elif ci == 2:
                            tail_affine(pending)
                        elif ci == 3:
                            tail_vpool(pending)
                        elif ci == 4:
                            tail_hpool(pending)
                        elif ci == 6:
                            tail_store(pending)
                            pending = None

                pending = {"b": b, "s1c": s1cols, "sqc": sqcols, "y": y_raw,
                           "st": stats, "n_act": n_act, "n_bn": n_bn,
                           "n_act_els": n_act_els, "pf": pool_first}
                if pool_first:
                    pending["hx"], pending["hn"] = hx, hn
            tail_stats(pending)
            tail_gsum(pending)
            tail_combine(pending)
            tail_store(pending)
    nc.finalize()
    _CACHED["nc"] = nc
    return nc


def _prep_consts(conv_w, conv_b, gn_w, gn_b, scale):
    # wp[kw, ci + 64*kh, co] = conv_w[co, ci, kh, kw] for kh in {0,1}
    # w2[ci, co] = conv_w[co, ci, 2, 0]; w2[64+ci, co] = conv_w[co, ci, 2, 1]
    # ws[ci, co] = conv_w[co, ci, 2, 2]
    w = np.ascontiguousarray(conv_w.astype(np.float32))
    wp = np.empty((128, 3 * COUT), np.float16)
    w2 = np.empty((128, COUT), np.float16)
    ws = np.empty((128, COUT), np.float16)
    for kw in range(3):
        wp[0:64, kw * COUT : (kw + 1) * COUT] = w[:, :, 0, kw].T
        wp[64:128, kw * COUT : (kw + 1) * COUT] = w[:, :, 1, kw].T
    w2[0:64, :] = w[:, :, 2, 0].T
    w2[64:128, :] = w[:, :, 2, 1].T
    ws[0:64, :] = w[:, :, 2, 2].T
    ws[64:128, :] = w[:, :, 2, 2].T
    cb = conv_b.astype(np.float32).reshape(COUT, 1)
    sc = scale.astype(np.float32).reshape(COUT)
    gs = (gn_w.astype(np.float32) * sc).reshape(COUT, 1)
    gbs = (gn_b.astype(np.float32) * sc).reshape(COUT, 1)
    bones = np.zeros((COUT, COUT), np.float32)
    for g in range(NG):
        bones[g * GSZ : (g + 1) * GSZ, g * GSZ : (g + 1) * GSZ] = 1.0
    return wp, w2, ws, cb, gs, gbs, bones


def kernel(x, conv_w, conv_b, gn_w, gn_b, scale):
    x = np.asarray(x, dtype=np.float32).astype(np.float16)
    wp, w2, ws, cb, gs, gbs, bones = _prep_consts(
        np.asarray(conv_w), np.asarray(conv_b), np.asarray(gn_w),
        np.asarray(gn_b), np.asarray(scale),
    )
    nc = _build()
    in_maps = []
    for c in range(N_CORES):
        in_maps.append({
            "xs": x[c * BPC : (c + 1) * BPC],
            "wp": wp, "w2": w2, "ws": ws,
            "cb": cb, "gs": gs, "gbs": gbs, "bones": bones,
        })
    results = _run_cached(nc, in_maps)
    out = np.concatenate([results[c]["out"] for c in range(N_CORES)], axis=0)
    return out.astype(np.float32)


def _run_cached(nc, in_maps):
    """run_bass_kernel_spmd's axon path with the jitted executable cached
    across calls (avoids re-tracing the shard_map wrapper every call)."""
    import jax
    import numpy as _np
    from jax.sharding import Mesh, PartitionSpec
    from jax.experimental.shard_map import shard_map
    from concourse import bass2jax

    if "runner" not in _CACHED:
        bass2jax.install_neuronx_cc_hook()
        partition_name = (
            nc.partition_id_tensor.name if nc.partition_id_tensor else None
        )
        in_names, out_names, out_avals, zero_outs = [], [], [], []
        for alloc in nc.m.functions[0].allocations:
            if not isinstance(alloc, mybir.MemoryLocationSet):
                continue
            name = alloc.memorylocations[0].name
            if alloc.kind == "ExternalInput":
                if name != partition_name:
                    in_names.append(name)
            elif alloc.kind == "ExternalOutput":
                shape = tuple(alloc.tensor_shape)
                dtype = mybir.dt.np(alloc.dtype)
                out_names.append(name)
                out_avals.append(jax.core.ShapedArray(shape, dtype))
                zero_outs.append(_np.zeros(shape, dtype))
        n_params = len(in_names)
        n_outs = len(out_avals)
        all_names = list(in_names) + list(out_names)
        if partition_name is not None:
            all_names.append(partition_name)
        donate = tuple(range(n_params, n_params + n_outs))

        def _body(*args):
            operands = list(args)
            if partition_name is not None:
                operands.append(bass2jax.partition_id_tensor())
            outs = bass2jax._bass_exec_p.bind(
                *operands,
                out_avals=tuple(out_avals),
                in_names=tuple(all_names),
                out_names=tuple(out_names),
                lowering_input_output_aliases=(),
                sim_require_finite=True,
                sim_require_nnan=True,
                nc=nc,
            )
            return tuple(outs)

        devices = jax.devices()[:N_CORES]
        mesh = Mesh(_np.asarray(devices), ("core",))
        in_specs = (PartitionSpec("core"),) * (n_params + n_outs)
        out_specs = (PartitionSpec("core"),) * n_outs
        sharded = jax.jit(
            shard_map(_body, mesh=mesh, in_specs=in_specs,
                      out_specs=out_specs, check_rep=False),
            donate_argnums=donate, keep_unused=True,
        )
        _CACHED["runner"] = (sharded, in_names, out_names, out_avals, zero_outs)

    sharded, in_names, out_names, out_avals, zero_outs = _CACHED["runner"]
    import numpy as _np2
    concat_in = [
        _np2.concatenate([_np2.asarray(in_maps[c][n]) for c in range(N_CORES)], axis=0)
        for n in in_names
    ]
    concat_zeros = [
        _np2.zeros((N_CORES * z.shape[0], *z.shape[1:]), z.dtype) for z in zero_outs
    ]
    out_arrs = sharded(*concat_in, *concat_zeros)
    return [
        {
            name: _np2.asarray(out_arrs[i]).reshape(N_CORES, *out_avals[i].shape)[c]
            for i, name in enumerate(out_names)
        }
        for c in range(N_CORES)
    ]


if __name__ == "__main__":
    rng = np.random.default_rng(0)
    x = rng.standard_normal((B_FULL, CIN, H, W), dtype=np.float32)
    cw = rng.standard_normal((COUT, CIN, 3, 3), dtype=np.float32)
    out = kernel(x, cw, rng.standard_normal(COUT, dtype=np.float32),
                 rng.standard_normal(COUT, dtype=np.float32),
                 rng.standard_normal(COUT, dtype=np.float32),
                 rng.standard_normal((COUT, 1, 1), dtype=np.float32))
    print(out.shape, out.dtype)


# revision 31
# speedup vs baseline: 1.0726x; 1.0166x over previous
"""Fused conv3x3 -> GroupNorm(16) -> channel scale -> maxpool2x2 -> clamp[0,1]
Trainium2 Bass kernel, data-parallel over batch on 8 NeuronCores.

Input  x [32, 64, 128, 128] f32  -> output [32, 128, 63, 63] f32.
Each core handles 4 samples.

Conv: fp16 tap-pair matmuls, 5 PE passes per 8-row output group half
(vs 9 naive):
  - xt buffer: partition ci holds x[ci, row], ci+64 holds x[ci, row+1]
    -> one [128,128] stacked weight covers taps (kh0,kw)+(kh1,kw): 3 passes
  - xq buffer: both blocks hold x[ci, row+2], block1 shifted 1 column
    (loaded as flat row-major slices at +0/+1 element offsets, so both are
    single contiguous DMAs) -> taps (kh2,0)+(kh2,1) in 1 pass; (kh2,2)
    reads xq block0 at column offset 2 as a 64-contraction pass.
The two 64-contraction passes land on disjoint PE row groups so their
execution overlaps; PE busy ~157us/core is within ~20% of the MAC roofline.

Tail strategy (keeps the PE fed; DVE and ACT each stay under the PE's
~39us/sample):
  - stats WITHOUT bn_stats: the ACT PSUM-evacuation Copy produces per-group
    sum(y) via accum_out; one DVE tensor_tensor_reduce per group (y*y ->
    scratch, accum add) produces sum(y^2). Conv bias is folded analytically.
  - affine BEFORE pooling, as a single in-place DVE tensor_scalar over the
    whole sample -- contiguous fp16 SBUF operands hit the DVE 4x perf mode
    (~0.26 ns/el), so this is 3x cheaper than splitting affine over ACT+DVE.
  - single max-pool branch, vertical pairs first (contiguous innermost ->
    DVE 2x mode), then horizontal pairs (strided, 1x), then fused clamp.
  - per-sample tails (coeff chain / affine / pools / store) are emitted
    interleaved with the next sample's conv chunks so no queue head-blocks.
fp16 output upcast to f32 on host.
"""

import numpy as np

import concourse.bacc as bacc
import concourse.mybir as mybir
import concourse.tile as tile
from concourse.bass_utils import run_bass_kernel_spmd

N_CORES = 8
B_FULL, CIN, H, W = 32, 64, 128, 128
COUT = 128
BPC = B_FULL // N_CORES  # samples per core
OH = OW = 126
PH = PW = 63
NG = 16  # groups
GSZ = COUT // NG  # 8 channels per group
EPS = 1e-5
S = OH * OW  # spatial size per sample
NEL = float(S)  # elements per channel for stats

# (x_row0, n_xrows, out_row0, n_out_rows)
CHUNKS = [(0, 10, 0, 8), (8, 10, 8, 8), (16, 10, 16, 8), (24, 14, 24, 12),
          (36, 18, 36, 16), (52, 24, 52, 22), (74, 24, 74, 22), (96, 24, 96, 22),
          (118, 10, 118, 8)]
XROWS_MAX = 24
NGROUPS = 17  # total 8-or-smaller row groups per sample
# row count per group (CHUNKS split into <=8-row groups)
GROUP_ROWS = [8, 8, 8, 8, 4, 8, 8, 8, 8, 6, 8, 8, 6, 8, 8, 6, 8]
# groups whose stats run on ACT (Square+accum); rest use DVE bn_stats.
# the last sample shifts most groups to ACT: its DVE also carries the
# previous sample's tail plus the last-sample max+min pools, and any DVE
# backlog there directly lengthens the serial post-conv tail
A6 = frozenset({1, 4, 7, 10, 13, 16})
A12 = frozenset(range(NGROUPS)) - {0, 4, 8, 12, 16}
ACT_SETS = [A6, A6, A12, A12]
# samples 2,3 pool during their own conv (max+min branches) and get a short
# all-DVE affine-combine tail; samples 0,1 use the cheaper affine-first tail
POOL_FIRST = (False, False, True, True)
N_ACT_MAX = max(len(s) for s in ACT_SETS)
N_BN_MAX = max(NGROUPS - len(s) for s in ACT_SETS)

_CACHED = {}


def _build():
    if "nc" in _CACHED:
        return _CACHED["nc"]
    f32 = mybir.dt.float32
    f16 = mybir.dt.float16
    AF = mybir.ActivationFunctionType
    OP = mybir.AluOpType

    nc = bacc.Bacc("TRN2", target_bir_lowering=False, debug=False)
    xs = nc.dram_tensor("xs", [BPC, CIN, H, W], f16, kind="ExternalInput").ap()
    wp_d = nc.dram_tensor("wp", [128, 3 * COUT], f16, kind="ExternalInput").ap()
    w2_d = nc.dram_tensor("w2", [128, COUT], f16, kind="ExternalInput").ap()
    ws_d = nc.dram_tensor("ws", [128, COUT], f16, kind="ExternalInput").ap()
    cb_d = nc.dram_tensor("cb", [COUT, 1], f32, kind="ExternalInput").ap()
    gs_d = nc.dram_tensor("gs", [COUT, 1], f32, kind="ExternalInput").ap()
    gbs_d = nc.dram_tensor("gbs", [COUT, 1], f32, kind="ExternalInput").ap()
    bones_d = nc.dram_tensor("bones", [COUT, COUT], f16, kind="ExternalInput").ap()
    out_d = nc.dram_tensor("out", [BPC, COUT, PH, PW], f16, kind="ExternalOutput").ap()

    with tile.TileContext(nc) as tc:
        with (
            tc.tile_pool(name="consts", bufs=1) as cpool,
            tc.tile_pool(name="xpool", bufs=3) as xpool,
            tc.tile_pool(name="xqpool", bufs=3) as xqpool,
            tc.tile_pool(name="ypool", bufs=2) as ypool,
            tc.tile_pool(name="sqpool", bufs=1) as sqpool,
            tc.tile_pool(name="stpool", bufs=2) as stpool,
            tc.tile_pool(name="pvpool", bufs=2) as pvpool,
            tc.tile_pool(name="popool", bufs=2) as popool,
            tc.tile_pool(name="vspool", bufs=2) as vspool,
            tc.tile_pool(name="hxpool", bufs=2) as hxpool,
            tc.tile_pool(name="cps", bufs=3, space="PSUM") as cps,
            tc.tile_pool(name="gps", bufs=1, space="PSUM") as gps,
        ):
            wp = cpool.tile([128, 3 * COUT], f16, name="wp_t")
            w2 = cpool.tile([128, COUT], f16, name="w2_t")
            ws = cpool.tile([128, COUT], f16, name="ws_t")
            cb = cpool.tile([COUT, 1], f32, name="cb_t")
            gs = cpool.tile([COUT, 1], f32, name="gs_t")
            gbs = cpool.tile([COUT, 1], f32, name="gbs_t")
            bones = cpool.tile([COUT, COUT], f16, name="bones_t")
            zeros1 = cpool.tile([COUT, 1], f32, name="zeros1")
            nc.vector.memset(zeros1[:], 0.0)

            def load_chunk(b, xr0, nor):
                xt = xpool.tile([128, XROWS_MAX, W], f16, tag="x", name="xt")
                nc.sync.dma_start(
                    xt[0:64, 0:nor, :], xs[b, :, xr0 : xr0 + nor, :]
                )
                nc.sync.dma_start(
                    xt[64:128, 0:nor, :], xs[b, :, xr0 + 1 : xr0 + 1 + nor, :]
                )
                xq = xqpool.tile([128, XROWS_MAX, W], f16, tag="xq", name="xq")
                xf = xs[b].rearrange("c h w -> c (h w)")
                off = (xr0 + 2) * W
                nc.sync.dma_start(
                    xq[0:64, 0:nor, :].rearrange("p a b -> p (a b)"),
                    xf[:, off : off + nor * W],
                )
                n2 = min(nor * W, H * W - off - 1)
                nc.sync.dma_start(
                    xq[64:128, 0:nor, :].rearrange("p a b -> p (a b)")[:, 0:n2],
                    xf[:, off + 1 : off + 1 + n2],
                )
                return xt, xq

            # first matmul needs wp + chunk-0 x: issue those DMAs first, the
            # remaining consts (not needed until later matmuls / tails) after
            nc.sync.dma_start(wp[:], wp_d[:])
            prefetch = load_chunk(0, CHUNKS[0][0], CHUNKS[0][3])
            nc.sync.dma_start(w2[:], w2_d[:])
            nc.sync.dma_start(ws[:], ws_d[:])
            nc.sync.dma_start(cb[:], cb_d[:])
            nc.sync.dma_start(gs[:], gs_d[:])
            nc.sync.dma_start(gbs[:], gbs_d[:])
            nc.sync.dma_start(bones[:], bones_d[:])
            # scratch for the ACT Square main output (discarded)
            sqscr = sqpool.tile([128, 8 * OW], f16, name="sqscr")

            def tail_stats(tl):
                # merge bn_stats Welford aggregate (N_BN_ELS els) with the
                # ACT-square raw sums (N_ACT_ELS els) into full-sample
                # st0 = E[y+cb], st1 = E[(y+cb)^2]
                n_bn_els = NEL - tl["n_act_els"]
                mv = stpool.tile([128, 2], f32, tag="mv", name="mv")
                nc.vector.bn_aggr(mv[:], tl["st"][:, 0 : 2 * tl["n_bn"], :])
                S1 = stpool.tile([128, 1], f32, tag="S1", name="S1")
                nc.vector.tensor_reduce(S1[:], tl["s1c"][:, 0 : tl["n_act"]],
                                        mybir.AxisListType.XYZW, OP.add)
                S2 = stpool.tile([128, 1], f32, tag="S2", name="S2")
                nc.vector.tensor_reduce(S2[:], tl["sqc"][:, 0 : tl["n_act"]],
                                        mybir.AxisListType.XYZW, OP.add)
                # S1 <- S1 + mean_a * N_a ; S2 <- S2 + (var_a+mean_a^2) * N_a
                t0 = stpool.tile([128, 1], f32, tag="t0", name="t0")
                nc.vector.tensor_scalar(t0[:], mv[:, 0:1], n_bn_els, S1[:],
                                        OP.mult, OP.add)
                t2 = stpool.tile([128, 1], f32, tag="t2", name="t2")
                nc.vector.tensor_tensor(t2[:], mv[:, 0:1], mv[:, 0:1], OP.mult)
                nc.vector.tensor_tensor(t2[:], t2[:], mv[:, 1:2], OP.add)
                nc.vector.tensor_scalar(t2[:], t2[:], n_bn_els, S2[:],
                                        OP.mult, OP.add)
                st = stpool.tile([128, 2], f32, tag="sts", name="st")
                # st0 = S1/N + cb ; st1 = S2/N + cb*(2*S1/N + cb)
                nc.vector.tensor_scalar(st[:, 0:1], t0[:], 1.0 / NEL, cb[:],
                                        OP.mult, OP.add)
                t1 = stpool.tile([128, 1], f32, tag="t1", name="t1")
                nc.vector.tensor_scalar(t1[:], t0[:], 2.0 / NEL, cb[:],
                                        OP.mult, OP.add)
                nc.vector.tensor_tensor(t1[:], t1[:], cb[:], OP.mult)
                nc.vector.tensor_scalar(st[:, 1:2], t2[:], 1.0 / NEL, None,
                                        OP.mult)
                nc.vector.tensor_tensor(st[:, 1:2], st[:, 1:2], t1[:], OP.add)
                stf = stpool.tile([128, 2], f16, tag="stf", name="stf")
                nc.vector.tensor_scalar(stf[:], st[:], 1.0, None, OP.mult)
                tl["stv"] = stf

            def tail_gsum(tl):
                st = tl["stv"]
                gsum = gps.tile([128, 2], f32, tag="gsum", name="gsum")
                nc.tensor.matmul(gsum[:], bones[:], st[:], start=True, stop=True)
                mgrp = stpool.tile([128, 1], f32, tag="mgrp", name="mgrp")
                nc.vector.tensor_scalar(
                    mgrp[:], gsum[:, 0:1], 1.0 / GSZ, None, OP.mult
                )
                vgrp = stpool.tile([128, 1], f32, tag="vgrp", name="vgrp")
                nc.vector.tensor_scalar(
                    vgrp[:], gsum[:, 1:2], 1.0 / GSZ, EPS, OP.mult, OP.add
                )
                msq = stpool.tile([128, 1], f32, tag="msq", name="msq")
                nc.vector.tensor_tensor(msq[:], mgrp[:], mgrp[:], OP.mult)
                nc.vector.tensor_tensor(vgrp[:], vgrp[:], msq[:], OP.subtract)
                sdev = stpool.tile([128, 1], f32, tag="sdev", name="sdev")
                nc.scalar.activation(sdev[:], vgrp[:], AF.Sqrt, bias=zeros1[:])
                inv = stpool.tile([128, 1], f32, tag="inv", name="inv")
                nc.vector.reciprocal(inv[:], sdev[:])
                Acoef = stpool.tile([128, 1], f32, tag="Ac", name="Acoef")
                nc.vector.tensor_tensor(Acoef[:], inv[:], gs[:], OP.mult)
                Bcoef = stpool.tile([128, 1], f32, tag="Bc", name="Bcoef")
                nc.vector.tensor_tensor(Bcoef[:], cb[:], mgrp[:], OP.subtract)
                nc.vector.tensor_tensor(Bcoef[:], Bcoef[:], Acoef[:], OP.mult)
                nc.vector.tensor_tensor(Bcoef[:], Bcoef[:], gbs[:], OP.add)
                tl["A"], tl["B"] = Acoef, Bcoef

            def tail_affine(tl):
                # z = A*y + B in place, whole sample (DVE 4x mode)
                y = tl["y"]
                nc.vector.tensor_scalar(
                    y[:], y[:], tl["A"][:], tl["B"][:], OP.mult, OP.add
                )

            def tail_vpool(tl):
                # vertical max over row pairs: [126,126] -> [63,126]
                y3 = tl["y"][:].rearrange("p (a b) -> p a b", b=OW)
                pv = pvpool.tile([128, PH, OW], f16, tag="pv", name="pv")
                nc.vector.tensor_tensor(
                    pv[:], y3[:, 0:OH:2, :], y3[:, 1:OH:2, :], OP.max
                )
                tl["pv"] = pv

            def tail_hpool(tl):
                # horizontal max over col pairs + clamp: [63,126] -> [63,63]
                pv = tl["pv"]
                po = popool.tile([128, PH, PW], f16, tag="po", name="po")
                nc.vector.tensor_tensor(
                    po[:], pv[:, :, 0:OW:2], pv[:, :, 1:OW:2], OP.max
                )
                nc.vector.tensor_scalar(po[:], po[:], 1.0, 0.0, OP.min, OP.max)
                tl["po"] = po

            def tail_combine(tl, r0=0, r1=PH):
                # z = clamp(max(A*maxpool(y)+B, A*minpool(y)+B), 0, 1)
                # == clamp(maxpool(A*y+B)) for either sign of A
                Acoef, Bcoef = tl["A"], tl["B"]
                hx_, hn_ = tl["hx"], tl["hn"]
                if r0 == 0:
                    tl["po"] = popool.tile([128, PH, PW], f16, tag="po",
                                           name="po")
                po = tl["po"]
                nc.vector.tensor_scalar(
                    po[:, r0:r1, :], hx_[:, r0:r1, :], Acoef[:], Bcoef[:],
                    OP.mult, OP.add
                )
                nc.vector.tensor_scalar(
                    hn_[:, r0:r1, :], hn_[:, r0:r1, :], Acoef[:], Bcoef[:],
                    OP.mult, OP.add
                )
                nc.vector.tensor_tensor(po[:, r0:r1, :], po[:, r0:r1, :],
                                        hn_[:, r0:r1, :], OP.max)
                nc.vector.tensor_scalar(po[:, r0:r1, :], po[:, r0:r1, :],
                                        1.0, 0.0, OP.min, OP.max)

            def tail_store(tl, r0=0, r1=PH):
                # dispatch from the idle GpSimd SWDGE: the store waits on the
                # DVE clamp, and on either hardware DGE queue (Sync carries
                # x-loads, ACT carries PSUM evacs) it would head-block work
                # the PE depends on, starving it for ~5-10us per sample
                nc.gpsimd.dma_start(
                    out_d[tl["b"], :, r0:r1, :].rearrange("c h w -> c (h w)"),
                    tl["po"][:, r0:r1, :].rearrange("p a b -> p (a b)"),
                )

            pending = None
            for b in range(BPC):
                pool_first = POOL_FIRST[b]
                if pool_first:
                    hx = hxpool.tile([128, PH, PW], f16, tag="hx", name="hx")
                    hn = hxpool.tile([128, PH, PW], f16, tag="hn", name="hn")
                y_raw = ypool.tile([128, S], f16, tag="y", name="y_raw")
                act_set = ACT_SETS[b]
                n_act = len(act_set)
                n_bn = NGROUPS - n_act
                n_act_els = float(sum(GROUP_ROWS[g] for g in act_set) * OW)
                s1cols = stpool.tile([128, N_ACT_MAX], f32, tag="s1c", name="s1cols")
                sqcols = stpool.tile([128, N_ACT_MAX], f32, tag="sqc", name="sqcols")
                stats = stpool.tile([128, 2 * N_BN_MAX, 6], f32,
                                    tag="st", name="stats")

                gi = 0  # group index within sample
                aci = 0  # ACT-square group counter
                si = 0  # bn_stats slot counter
                for ci, (xr0, nxr, or0, nor) in enumerate(CHUNKS):
                    # xt block0 = x rows xr0.., block1 = x rows xr0+1..;
                    # only `nor` rows each are read (kh2 taps come from xq).
                    # xq holds x rows (xr0+2) duplicated with a 1-column shift
                    # between partition blocks -> covers taps (kh2,kw0)+(kh2,kw1)
                    # in one 128-contraction matmul; block0 also serves (kh2,kw2)
                    if b == 0 and ci == 0:
                        xt, xq = prefetch
                    else:
                        xt, xq = load_chunk(b, xr0, nor)

                    g0 = or0
                    while g0 < or0 + nor:
                        gn = min(8, or0 + nor - g0)  # 8, 6 or 4 output rows
                        hr = gn // 2  # rows per half
                        cp = cps.tile([128, 1024], f32, tag="cp", name="cp")
                        for half in range(2):
                            row0 = g0 + half * hr
                            l0 = row0 - xr0
                            outap = cp[:, half * 512 : half * 512 + hr * OW]
                            for kw in range(3):
                                nc.tensor.matmul(
                                    outap,
                                    wp[:, kw * COUT : (kw + 1) * COUT],
                                    xt[:, l0 : l0 + hr, kw : kw + OW],
                                    start=(kw == 0),
                                    stop=False,
                                )
                            nc.tensor.matmul(
                                outap,
                                w2[:],
                                xq[:, l0 : l0 + hr, 0:OW],
                                start=False,
                                stop=False,
                            )
                        # (kh2,kw2) singles for both halves, adjacent on
                        # disjoint PE row groups (0-63 / 64-127) so the
                        # 16x 32x32 sub-arrays overlap their execution.
                        # half1 reads xq block1 (data shifted +1 col) at
                        # offset 1 -> x column c+2, same tap.
                        l0a = g0 - xr0
                        l0b = g0 + hr - xr0
                        nc.tensor.matmul(
                            cp[:, 0 : hr * OW],
                            ws[0:64, :],
                            xq[0:64, l0a : l0a + hr, 2 : 2 + OW],
                            start=False,
                            stop=True,
                            skip_group_check=True,
                        )
                        nc.tensor.matmul(
                            cp[:, 512 : 512 + hr * OW],
                            ws[64:128, :],
                            xq[64:128, l0b : l0b + hr, 1 : 1 + OW],
                            start=False,
                            stop=True,
                            skip_group_check=True,
                        )
                        # evacuate both halves in one strided ACT copy;
                        # accum_out gives this group's per-channel sum(y)
                        yv = y_raw[:, g0 * OW : (g0 + gn) * OW].rearrange(
                            "p (a b) -> p a b", b=hr * OW
                        )
                        on_act = gi in act_set
                        nc.scalar.activation(
                            yv,
                            cp[:].rearrange("p (a b) -> p a b", b=512)[
                                :, :, 0 : hr * OW
                            ],
                            AF.Copy,
                            accum_out=(s1cols[:, aci : aci + 1]
                                       if on_act else None),
                        )
                        yseg = y_raw[:, g0 * OW : (g0 + gn) * OW]
                        if on_act:
                            # ACT square pass: accum gives sum(y^2)
                            nc.scalar.activation(
                                sqscr[:, 0 : gn * OW], yseg, AF.Square,
                                accum_out=sqcols[:, aci : aci + 1],
                            )
                            aci += 1
                        else:
                            # DVE one-pass Welford stats per 504-el half
                            for half in range(2):
                                r0 = (g0 + half * hr) * OW
                                nc.vector.bn_stats(
                                    stats[:, si, :],
                                    y_raw[:, r0 : r0 + hr * OW],
                                )
                                si += 1
                        gi += 1
                        g0 += gn

                    if pool_first:
                        # pool during the sample's own conv (max+min branches
                        # on raw y) so only a short affine-combine tail is left
                        y3c = y_raw[:].rearrange("p (a b) -> p a b", b=OW)
                        h0 = or0 // 2
                        nh = nor // 2
                        vx = vspool.tile([128, 11, OW], f16, tag="vx", name="vx")
                        vn = vspool.tile([128, 11, OW], f16, tag="vn", name="vn")
                        nc.vector.tensor_tensor(
                            vx[:, 0:nh, :],
                            y3c[:, or0 : or0 + nor : 2, :],
                            y3c[:, or0 + 1 : or0 + nor : 2, :],
                            OP.max,
                        )
                        nc.vector.tensor_tensor(
                            vn[:, 0:nh, :],
                            y3c[:, or0 : or0 + nor : 2, :],
                            y3c[:, or0 + 1 : or0 + nor : 2, :],
                            OP.min,
                        )
                        nc.vector.tensor_tensor(
                            hx[:, h0 : h0 + nh, :],
                            vx[:, 0:nh, 0 : OW : 2],
                            vx[:, 0:nh, 1 : OW : 2],
                            OP.max,
                        )
                        nc.vector.tensor_tensor(
                            hn[:, h0 : h0 + nh, :],
                            vn[:, 0:nh, 0 : OW : 2],
                            vn[:, 0:nh, 1 : OW : 2],
                            OP.min,
                        )
                    if pending is not None:
                        if ci == 0:
                            tail_stats(pending)
                        elif ci == 1:
                            tail_gsum(pending)
                        elif pending["pf"]:
                            if ci == 2:
                                tail_combine(pending)
                            elif ci == 3:
                                tail_store(pending)
                                pending = None
                        elif ci == 2:
                            tail_affine(pending)
                        elif ci == 3:
                            tail_vpool(pending)
                        elif ci == 4:
                            tail_hpool(pending)
                        elif ci == 6:
                            tail_store(pending)
                            pending = None

                pending = {"b": b, "s1c": s1cols, "sqc": sqcols, "y": y_raw,
                           "st": stats, "n_act": n_act, "n_bn": n_bn,
                           "n_act_els": n_act_els, "pf": pool_first}
                if pool_first:
                    pending["hx"], pending["hn"] = hx, hn
            tail_stats(pending)
            tail_gsum(pending)
            tail_combine(pending, 0, 32)
            tail_store(pending, 0, 32)
            tail_combine(pending, 32, PH)
            tail_store(pending, 32, PH)
    nc.finalize()
    _CACHED["nc"] = nc
    return nc


def _prep_consts(conv_w, conv_b, gn_w, gn_b, scale):
    # wp[kw, ci + 64*kh, co] = conv_w[co, ci, kh, kw] for kh in {0,1}
    # w2[ci, co] = conv_w[co, ci, 2, 0]; w2[64+ci, co] = conv_w[co, ci, 2, 1]
    # ws[ci, co] = conv_w[co, ci, 2, 2]
    w = np.ascontiguousarray(conv_w.astype(np.float32))
    wp = np.empty((128, 3 * COUT), np.float16)
    w2 = np.empty((128, COUT), np.float16)
    ws = np.empty((128, COUT), np.float16)
    for kw in range(3):
        wp[0:64, kw * COUT : (kw + 1) * COUT] = w[:, :, 0, kw].T
        wp[64:128, kw * COUT : (kw + 1) * COUT] = w[:, :, 1, kw].T
    w2[0:64, :] = w[:, :, 2, 0].T
    w2[64:128, :] = w[:, :, 2, 1].T
    ws[0:64, :] = w[:, :, 2, 2].T
    ws[64:128, :] = w[:, :, 2, 2].T
    cb = conv_b.astype(np.float32).reshape(COUT, 1)
    sc = scale.astype(np.float32).reshape(COUT)
    gs = (gn_w.astype(np.float32) * sc).reshape(COUT, 1)
    gbs = (gn_b.astype(np.float32) * sc).reshape(COUT, 1)
    bones = np.zeros((COUT, COUT), np.float16)
    for g in range(NG):
        bones[g * GSZ : (g + 1) * GSZ, g * GSZ : (g + 1) * GSZ] = 1.0
    return wp, w2, ws, cb, gs, gbs, bones


def kernel(x, conv_w, conv_b, gn_w, gn_b, scale):
    x = np.asarray(x, dtype=np.float32).astype(np.float16)
    wp, w2, ws, cb, gs, gbs, bones = _prep_consts(
        np.asarray(conv_w), np.asarray(conv_b), np.asarray(gn_w),
        np.asarray(gn_b), np.asarray(scale),
    )
    nc = _build()
    in_maps = []
    for c in range(N_CORES):
        in_maps.append({
            "xs": x[c * BPC : (c + 1) * BPC],
            "wp": wp, "w2": w2, "ws": ws,
            "cb": cb, "gs": gs, "gbs": gbs, "bones": bones,
        })
    results = _run_cached(nc, in_maps)
    out = np.concatenate([results[c]["out"] for c in range(N_CORES)], axis=0)
    return out.astype(np.float32)


def _run_cached(nc, in_maps):
    """run_bass_kernel_spmd's axon path with the jitted executable cached
    across calls (avoids re-tracing the shard_map wrapper every call)."""
    import jax
    import numpy as _np
    from jax.sharding import Mesh, PartitionSpec
    from jax.experimental.shard_map import shard_map
    from concourse import bass2jax

    if "runner" not in _CACHED:
        bass2jax.install_neuronx_cc_hook()
        partition_name = (
            nc.partition_id_tensor.name if nc.partition_id_tensor else None
        )
        in_names, out_names, out_avals, zero_outs = [], [], [], []
        for alloc in nc.m.functions[0].allocations:
            if not isinstance(alloc, mybir.MemoryLocationSet):
                continue
            name = alloc.memorylocations[0].name
            if alloc.kind == "ExternalInput":
                if name != partition_name:
                    in_names.append(name)
            elif alloc.kind == "ExternalOutput":
                shape = tuple(alloc.tensor_shape)
                dtype = mybir.dt.np(alloc.dtype)
                out_names.append(name)
                out_avals.append(jax.core.ShapedArray(shape, dtype))
                zero_outs.append(_np.zeros(shape, dtype))
        n_params = len(in_names)
        n_outs = len(out_avals)
        all_names = list(in_names) + list(out_names)
        if partition_name is not None:
            all_names.append(partition_name)
        donate = tuple(range(n_params, n_params + n_outs))

        def _body(*args):
            operands = list(args)
            if partition_name is not None:
                operands.append(bass2jax.partition_id_tensor())
            outs = bass2jax._bass_exec_p.bind(
                *operands,
                out_avals=tuple(out_avals),
                in_names=tuple(all_names),
                out_names=tuple(out_names),
                lowering_input_output_aliases=(),
                sim_require_finite=True,
                sim_require_nnan=True,
                nc=nc,
            )
            return tuple(outs)

        devices = jax.devices()[:N_CORES]
        mesh = Mesh(_np.asarray(devices), ("core",))
        in_specs = (PartitionSpec("core"),) * (n_params + n_outs)
        out_specs = (PartitionSpec("core"),) * n_outs
        sharded = jax.jit(
            shard_map(_body, mesh=mesh, in_specs=in_specs,
                      out_specs=out_specs, check_rep=False),
            donate_argnums=donate, keep_unused=True,
        )
        _CACHED["runner"] = (sharded, in_names, out_names, out_avals, zero_outs)

    sharded, in_names, out_names, out_avals, zero_outs = _CACHED["runner"]
    import numpy as _np2
    concat_in = [
        _np2.concatenate([_np2.asarray(in_maps[c][n]) for c in range(N_CORES)], axis=0)
        for n in in_names
    ]
    concat_zeros = [
        _np2.zeros((N_CORES * z.shape[0], *z.shape[1:]), z.dtype) for z in zero_outs
    ]
    out_arrs = sharded(*concat_in, *concat_zeros)
    return [
        {
            name: _np2.asarray(out_arrs[i]).reshape(N_CORES, *out_avals[i].shape)[c]
            for i, name in enumerate(out_names)
        }
        for c in range(N_CORES)
    ]


if __name__ == "__main__":
    rng = np.random.default_rng(0)
    x = rng.standard_normal((B_FULL, CIN, H, W), dtype=np.float32)
    cw = rng.standard_normal((COUT, CIN, 3, 3), dtype=np.float32)
    out = kernel(x, cw, rng.standard_normal(COUT, dtype=np.float32),
                 rng.standard_normal(COUT, dtype=np.float32),
                 rng.standard_normal(COUT, dtype=np.float32),
                 rng.standard_normal((COUT, 1, 1), dtype=np.float32))
    print(out.shape, out.dtype)


# revision 32
# speedup vs baseline: 1.0829x; 1.0096x over previous
"""Fused conv3x3 -> GroupNorm(16) -> channel scale -> maxpool2x2 -> clamp[0,1]
Trainium2 Bass kernel, data-parallel over batch on 8 NeuronCores.

Input  x [32, 64, 128, 128] f32  -> output [32, 128, 63, 63] f32.
Each core handles 4 samples.

Conv: fp16 tap-pair matmuls, 5 PE passes per 8-row output group half
(vs 9 naive):
  - xt buffer: partition ci holds x[ci, row], ci+64 holds x[ci, row+1]
    -> one [128,128] stacked weight covers taps (kh0,kw)+(kh1,kw): 3 passes
  - xq buffer: both blocks hold x[ci, row+2], block1 shifted 1 column
    (loaded as flat row-major slices at +0/+1 element offsets, so both are
    single contiguous DMAs) -> taps (kh2,0)+(kh2,1) in 1 pass; (kh2,2)
    reads xq block0 at column offset 2 as a 64-contraction pass.
The two 64-contraction passes land on disjoint PE row groups so their
execution overlaps; PE busy ~157us/core is within ~20% of the MAC roofline.

Tail strategy (keeps the PE fed; DVE and ACT each stay under the PE's
~39us/sample):
  - stats WITHOUT bn_stats: the ACT PSUM-evacuation Copy produces per-group
    sum(y) via accum_out; one DVE tensor_tensor_reduce per group (y*y ->
    scratch, accum add) produces sum(y^2). Conv bias is folded analytically.
  - affine BEFORE pooling, as a single in-place DVE tensor_scalar over the
    whole sample -- contiguous fp16 SBUF operands hit the DVE 4x perf mode
    (~0.26 ns/el), so this is 3x cheaper than splitting affine over ACT+DVE.
  - single max-pool branch, vertical pairs first (contiguous innermost ->
    DVE 2x mode), then horizontal pairs (strided, 1x), then fused clamp.
  - per-sample tails (coeff chain / affine / pools / store) are emitted
    interleaved with the next sample's conv chunks so no queue head-blocks.
fp16 output upcast to f32 on host.
"""

import numpy as np

import concourse.bacc as bacc
import concourse.mybir as mybir
import concourse.tile as tile
from concourse.bass_utils import run_bass_kernel_spmd

N_CORES = 8
B_FULL, CIN, H, W = 32, 64, 128, 128
COUT = 128
BPC = B_FULL // N_CORES  # samples per core
OH = OW = 126
PH = PW = 63
NG = 16  # groups
GSZ = COUT // NG  # 8 channels per group
EPS = 1e-5
S = OH * OW  # spatial size per sample
NEL = float(S)  # elements per channel for stats

# (x_row0, n_xrows, out_row0, n_out_rows)
CHUNKS = [(0, 10, 0, 8), (8, 10, 8, 8), (16, 10, 16, 8), (24, 14, 24, 12),
          (36, 18, 36, 16), (52, 24, 52, 22), (74, 24, 74, 22), (96, 24, 96, 22),
          (118, 10, 118, 8)]
XROWS_MAX = 24
NGROUPS = 17  # total 8-or-smaller row groups per sample
# row count per group (CHUNKS split into <=8-row groups)
GROUP_ROWS = [8, 8, 8, 8, 4, 8, 8, 8, 8, 6, 8, 8, 6, 8, 8, 6, 8]
# groups whose stats run on ACT (Square+accum); rest use DVE bn_stats.
# the last sample shifts most groups to ACT: its DVE also carries the
# previous sample's tail plus the last-sample max+min pools, and any DVE
# backlog there directly lengthens the serial post-conv tail
A6 = frozenset({1, 4, 7, 10, 13, 16})
A12 = frozenset(range(NGROUPS)) - {0, 4, 8, 12, 16}
ACT_SETS = [A6, A6, A12, A12]
# samples 2,3 pool during their own conv (max+min branches) and get a short
# all-DVE affine-combine tail; samples 0,1 use the cheaper affine-first tail
POOL_FIRST = (False, False, True, True)
N_ACT_MAX = max(len(s) for s in ACT_SETS)
N_BN_MAX = max(NGROUPS - len(s) for s in ACT_SETS)

_CACHED = {}


def _build():
    if "nc" in _CACHED:
        return _CACHED["nc"]
    f32 = mybir.dt.float32
    f16 = mybir.dt.float16
    AF = mybir.ActivationFunctionType
    OP = mybir.AluOpType

    nc = bacc.Bacc("TRN2", target_bir_lowering=False, debug=False)
    xs = nc.dram_tensor("xs", [BPC, CIN, H, W], f16, kind="ExternalInput").ap()
    wp_d = nc.dram_tensor("wp", [128, 3 * COUT], f16, kind="ExternalInput").ap()
    w2_d = nc.dram_tensor("w2", [128, COUT], f16, kind="ExternalInput").ap()
    ws_d = nc.dram_tensor("ws", [128, COUT], f16, kind="ExternalInput").ap()
    cb_d = nc.dram_tensor("cb", [COUT, 1], f32, kind="ExternalInput").ap()
    gs_d = nc.dram_tensor("gs", [COUT, 1], f32, kind="ExternalInput").ap()
    gbs_d = nc.dram_tensor("gbs", [COUT, 1], f32, kind="ExternalInput").ap()
    bones_d = nc.dram_tensor("bones", [COUT, COUT], f16, kind="ExternalInput").ap()
    out_d = nc.dram_tensor("out", [BPC, COUT, PH, PW], f16, kind="ExternalOutput").ap()

    with tile.TileContext(nc) as tc:
        with (
            tc.tile_pool(name="consts", bufs=1) as cpool,
            tc.tile_pool(name="xpool", bufs=4) as xpool,
            tc.tile_pool(name="xqpool", bufs=3) as xqpool,
            tc.tile_pool(name="ypool", bufs=2) as ypool,
            tc.tile_pool(name="sqpool", bufs=1) as sqpool,
            tc.tile_pool(name="stpool", bufs=2) as stpool,
            tc.tile_pool(name="pvpool", bufs=2) as pvpool,
            tc.tile_pool(name="popool", bufs=2) as popool,
            tc.tile_pool(name="vspool", bufs=1) as vspool,
            tc.tile_pool(name="hxpool", bufs=2) as hxpool,
            tc.tile_pool(name="cps", bufs=3, space="PSUM") as cps,
            tc.tile_pool(name="gps", bufs=1, space="PSUM") as gps,
        ):
            wp = cpool.tile([128, 3 * COUT], f16, name="wp_t")
            w2 = cpool.tile([128, COUT], f16, name="w2_t")
            ws = cpool.tile([128, COUT], f16, name="ws_t")
            cb = cpool.tile([COUT, 1], f32, name="cb_t")
            gs = cpool.tile([COUT, 1], f32, name="gs_t")
            gbs = cpool.tile([COUT, 1], f32, name="gbs_t")
            bones = cpool.tile([COUT, COUT], f16, name="bones_t")
            zeros1 = cpool.tile([COUT, 1], f32, name="zeros1")
            nc.vector.memset(zeros1[:], 0.0)

            def load_chunk(b, xr0, nor):
                xt = xpool.tile([128, XROWS_MAX, W], f16, tag="x", name="xt")
                nc.sync.dma_start(
                    xt[0:64, 0:nor, :], xs[b, :, xr0 : xr0 + nor, :]
                )
                nc.sync.dma_start(
                    xt[64:128, 0:nor, :], xs[b, :, xr0 + 1 : xr0 + 1 + nor, :]
                )
                xq = xqpool.tile([128, XROWS_MAX, W], f16, tag="xq", name="xq")
                xf = xs[b].rearrange("c h w -> c (h w)")
                off = (xr0 + 2) * W
                nc.sync.dma_start(
                    xq[0:64, 0:nor, :].rearrange("p a b -> p (a b)"),
                    xf[:, off : off + nor * W],
                )
                n2 = min(nor * W, H * W - off - 1)
                nc.sync.dma_start(
                    xq[64:128, 0:nor, :].rearrange("p a b -> p (a b)")[:, 0:n2],
                    xf[:, off + 1 : off + 1 + n2],
                )
                return xt, xq

            # first matmul needs wp + chunk-0 x: issue those DMAs first, the
            # remaining consts (not needed until later matmuls / tails) after
            nc.sync.dma_start(wp[:], wp_d[:])
            prefetch = load_chunk(0, CHUNKS[0][0], CHUNKS[0][3])
            nc.sync.dma_start(w2[:], w2_d[:])
            nc.sync.dma_start(ws[:], ws_d[:])
            nc.sync.dma_start(cb[:], cb_d[:])
            nc.sync.dma_start(gs[:], gs_d[:])
            nc.sync.dma_start(gbs[:], gbs_d[:])
            nc.sync.dma_start(bones[:], bones_d[:])
            # scratch for the ACT Square main output (discarded)
            sqscr = sqpool.tile([128, 8 * OW], f16, name="sqscr")

            def tail_stats(tl):
                # merge bn_stats Welford aggregate (N_BN_ELS els) with the
                # ACT-square raw sums (N_ACT_ELS els) into full-sample
                # st0 = E[y+cb], st1 = E[(y+cb)^2]
                n_bn_els = NEL - tl["n_act_els"]
                mv = stpool.tile([128, 2], f32, tag="mv", name="mv")
                nc.vector.bn_aggr(mv[:], tl["st"][:, 0 : 2 * tl["n_bn"], :])
                S1 = stpool.tile([128, 1], f32, tag="S1", name="S1")
                nc.vector.tensor_reduce(S1[:], tl["s1c"][:, 0 : tl["n_act"]],
                                        mybir.AxisListType.XYZW, OP.add)
                S2 = stpool.tile([128, 1], f32, tag="S2", name="S2")
                nc.vector.tensor_reduce(S2[:], tl["sqc"][:, 0 : tl["n_act"]],
                                        mybir.AxisListType.XYZW, OP.add)
                # S1 <- S1 + mean_a * N_a ; S2 <- S2 + (var_a+mean_a^2) * N_a
                t0 = stpool.tile([128, 1], f32, tag="t0", name="t0")
                nc.vector.tensor_scalar(t0[:], mv[:, 0:1], n_bn_els, S1[:],
                                        OP.mult, OP.add)
                t2 = stpool.tile([128, 1], f32, tag="t2", name="t2")
                nc.vector.tensor_tensor(t2[:], mv[:, 0:1], mv[:, 0:1], OP.mult)
                nc.vector.tensor_tensor(t2[:], t2[:], mv[:, 1:2], OP.add)
                nc.vector.tensor_scalar(t2[:], t2[:], n_bn_els, S2[:],
                                        OP.mult, OP.add)
                st = stpool.tile([128, 2], f32, tag="sts", name="st")
                # st0 = S1/N + cb ; st1 = S2/N + cb*(2*S1/N + cb)
                nc.vector.tensor_scalar(st[:, 0:1], t0[:], 1.0 / NEL, cb[:],
                                        OP.mult, OP.add)
                t1 = stpool.tile([128, 1], f32, tag="t1", name="t1")
                nc.vector.tensor_scalar(t1[:], t0[:], 2.0 / NEL, cb[:],
                                        OP.mult, OP.add)
                nc.vector.tensor_tensor(t1[:], t1[:], cb[:], OP.mult)
                nc.vector.tensor_scalar(st[:, 1:2], t2[:], 1.0 / NEL, None,
                                        OP.mult)
                nc.vector.tensor_tensor(st[:, 1:2], st[:, 1:2], t1[:], OP.add)
                stf = stpool.tile([128, 2], f16, tag="stf", name="stf")
                nc.vector.tensor_scalar(stf[:], st[:], 1.0, None, OP.mult)
                tl["stv"] = stf

            def tail_gsum(tl):
                st = tl["stv"]
                gsum = gps.tile([128, 2], f32, tag="gsum", name="gsum")
                nc.tensor.matmul(gsum[:], bones[:], st[:], start=True, stop=True)
                mgrp = stpool.tile([128, 1], f32, tag="mgrp", name="mgrp")
                nc.vector.tensor_scalar(
                    mgrp[:], gsum[:, 0:1], 1.0 / GSZ, None, OP.mult
                )
                vgrp = stpool.tile([128, 1], f32, tag="vgrp", name="vgrp")
                nc.vector.tensor_scalar(
                    vgrp[:], gsum[:, 1:2], 1.0 / GSZ, EPS, OP.mult, OP.add
                )
                msq = stpool.tile([128, 1], f32, tag="msq", name="msq")
                nc.vector.tensor_tensor(msq[:], mgrp[:], mgrp[:], OP.mult)
                nc.vector.tensor_tensor(vgrp[:], vgrp[:], msq[:], OP.subtract)
                sdev = stpool.tile([128, 1], f32, tag="sdev", name="sdev")
                nc.scalar.activation(sdev[:], vgrp[:], AF.Sqrt, bias=zeros1[:])
                inv = stpool.tile([128, 1], f32, tag="inv", name="inv")
                nc.vector.reciprocal(inv[:], sdev[:])
                Acoef = stpool.tile([128, 1], f32, tag="Ac", name="Acoef")
                nc.vector.tensor_tensor(Acoef[:], inv[:], gs[:], OP.mult)
                Bcoef = stpool.tile([128, 1], f32, tag="Bc", name="Bcoef")
                nc.vector.tensor_tensor(Bcoef[:], cb[:], mgrp[:], OP.subtract)
                nc.vector.tensor_tensor(Bcoef[:], Bcoef[:], Acoef[:], OP.mult)
                nc.vector.tensor_tensor(Bcoef[:], Bcoef[:], gbs[:], OP.add)
                tl["A"], tl["B"] = Acoef, Bcoef

            def tail_affine(tl):
                # z = A*y + B in place, whole sample (DVE 4x mode)
                y = tl["y"]
                nc.vector.tensor_scalar(
                    y[:], y[:], tl["A"][:], tl["B"][:], OP.mult, OP.add
                )

            def tail_vpool(tl):
                # vertical max over row pairs: [126,126] -> [63,126]
                y3 = tl["y"][:].rearrange("p (a b) -> p a b", b=OW)
                pv = pvpool.tile([128, PH, OW], f16, tag="pv", name="pv")
                nc.vector.tensor_tensor(
                    pv[:], y3[:, 0:OH:2, :], y3[:, 1:OH:2, :], OP.max
                )
                tl["pv"] = pv

            def tail_hpool(tl):
                # horizontal max over col pairs + clamp: [63,126] -> [63,63]
                pv = tl["pv"]
                po = popool.tile([128, PH, PW], f16, tag="po", name="po")
                nc.vector.tensor_tensor(
                    po[:], pv[:, :, 0:OW:2], pv[:, :, 1:OW:2], OP.max
                )
                nc.vector.tensor_scalar(po[:], po[:], 1.0, 0.0, OP.min, OP.max)
                tl["po"] = po

            def tail_combine(tl, r0=0, r1=PH):
                # z = clamp(max(A*maxpool(y)+B, A*minpool(y)+B), 0, 1)
                # == clamp(maxpool(A*y+B)) for either sign of A
                Acoef, Bcoef = tl["A"], tl["B"]
                hx_, hn_ = tl["hx"], tl["hn"]
                if r0 == 0:
                    tl["po"] = popool.tile([128, PH, PW], f16, tag="po",
                                           name="po")
                po = tl["po"]
                nc.vector.tensor_scalar(
                    po[:, r0:r1, :], hx_[:, r0:r1, :], Acoef[:], Bcoef[:],
                    OP.mult, OP.add
                )
                nc.vector.tensor_scalar(
                    hn_[:, r0:r1, :], hn_[:, r0:r1, :], Acoef[:], Bcoef[:],
                    OP.mult, OP.add
                )
                nc.vector.tensor_tensor(po[:, r0:r1, :], po[:, r0:r1, :],
                                        hn_[:, r0:r1, :], OP.max)
                nc.vector.tensor_scalar(po[:, r0:r1, :], po[:, r0:r1, :],
                                        1.0, 0.0, OP.min, OP.max)

            def tail_store(tl, r0=0, r1=PH):
                # dispatch from the idle GpSimd SWDGE: the store waits on the
                # DVE clamp, and on either hardware DGE queue (Sync carries
                # x-loads, ACT carries PSUM evacs) it would head-block work
                # the PE depends on, starving it for ~5-10us per sample
                nc.gpsimd.dma_start(
                    out_d[tl["b"], :, r0:r1, :].rearrange("c h w -> c (h w)"),
                    tl["po"][:, r0:r1, :].rearrange("p a b -> p (a b)"),
                )

            pending = None
            for b in range(BPC):
                pool_first = POOL_FIRST[b]
                if pool_first:
                    hx = hxpool.tile([128, PH, PW], f16, tag="hx", name="hx")
                    hn = hxpool.tile([128, PH, PW], f16, tag="hn", name="hn")
                y_raw = ypool.tile([128, S], f16, tag="y", name="y_raw")
                act_set = ACT_SETS[b]
                n_act = len(act_set)
                n_bn = NGROUPS - n_act
                n_act_els = float(sum(GROUP_ROWS[g] for g in act_set) * OW)
                s1cols = stpool.tile([128, N_ACT_MAX], f32, tag="s1c", name="s1cols")
                sqcols = stpool.tile([128, N_ACT_MAX], f32, tag="sqc", name="sqcols")
                stats = stpool.tile([128, 2 * N_BN_MAX, 6], f32,
                                    tag="st", name="stats")

                gi = 0  # group index within sample
                aci = 0  # ACT-square group counter
                si = 0  # bn_stats slot counter
                for ci, (xr0, nxr, or0, nor) in enumerate(CHUNKS):
                    # xt block0 = x rows xr0.., block1 = x rows xr0+1..;
                    # only `nor` rows each are read (kh2 taps come from xq).
                    # xq holds x rows (xr0+2) duplicated with a 1-column shift
                    # between partition blocks -> covers taps (kh2,kw0)+(kh2,kw1)
                    # in one 128-contraction matmul; block0 also serves (kh2,kw2)
                    if b == 0 and ci == 0:
                        xt, xq = prefetch
                    else:
                        xt, xq = load_chunk(b, xr0, nor)

                    g0 = or0
                    while g0 < or0 + nor:
                        gn = min(8, or0 + nor - g0)  # 8, 6 or 4 output rows
                        hr = gn // 2  # rows per half
                        cp = cps.tile([128, 1024], f32, tag="cp", name="cp")
                        for half in range(2):
                            row0 = g0 + half * hr
                            l0 = row0 - xr0
                            outap = cp[:, half * 512 : half * 512 + hr * OW]
                            for kw in range(3):
                                nc.tensor.matmul(
                                    outap,
                                    wp[:, kw * COUT : (kw + 1) * COUT],
                                    xt[:, l0 : l0 + hr, kw : kw + OW],
                                    start=(kw == 0),
                                    stop=False,
                                )
                            nc.tensor.matmul(
                                outap,
                                w2[:],
                                xq[:, l0 : l0 + hr, 0:OW],
                                start=False,
                                stop=False,
                            )
                        # (kh2,kw2) singles for both halves, adjacent on
                        # disjoint PE row groups (0-63 / 64-127) so the
                        # 16x 32x32 sub-arrays overlap their execution.
                        # half1 reads xq block1 (data shifted +1 col) at
                        # offset 1 -> x column c+2, same tap.
                        l0a = g0 - xr0
                        l0b = g0 + hr - xr0
                        nc.tensor.matmul(
                            cp[:, 0 : hr * OW],
                            ws[0:64, :],
                            xq[0:64, l0a : l0a + hr, 2 : 2 + OW],
                            start=False,
                            stop=True,
                            skip_group_check=True,
                        )
                        nc.tensor.matmul(
                            cp[:, 512 : 512 + hr * OW],
                            ws[64:128, :],
                            xq[64:128, l0b : l0b + hr, 1 : 1 + OW],
                            start=False,
                            stop=True,
                            skip_group_check=True,
                        )
                        # evacuate both halves in one strided ACT copy;
                        # accum_out gives this group's per-channel sum(y)
                        yv = y_raw[:, g0 * OW : (g0 + gn) * OW].rearrange(
                            "p (a b) -> p a b", b=hr * OW
                        )
                        on_act = gi in act_set
                        nc.scalar.activation(
                            yv,
                            cp[:].rearrange("p (a b) -> p a b", b=512)[
                                :, :, 0 : hr * OW
                            ],
                            AF.Copy,
                            accum_out=(s1cols[:, aci : aci + 1]
                                       if on_act else None),
                        )
                        yseg = y_raw[:, g0 * OW : (g0 + gn) * OW]
                        if on_act:
                            # ACT square pass: accum gives sum(y^2)
                            nc.scalar.activation(
                                sqscr[:, 0 : gn * OW], yseg, AF.Square,
                                accum_out=sqcols[:, aci : aci + 1],
                            )
                            aci += 1
                        else:
                            # DVE one-pass Welford stats per 504-el half
                            for half in range(2):
                                r0 = (g0 + half * hr) * OW
                                nc.vector.bn_stats(
                                    stats[:, si, :],
                                    y_raw[:, r0 : r0 + hr * OW],
                                )
                                si += 1
                        gi += 1
                        g0 += gn

                    if pool_first:
                        # pool during the sample's own conv (max+min branches
                        # on raw y) so only a short affine-combine tail is left
                        y3c = y_raw[:].rearrange("p (a b) -> p a b", b=OW)
                        h0 = or0 // 2
                        nh = nor // 2
                        vx = vspool.tile([128, 11, OW], f16, tag="vx", name="vx")
                        vn = vspool.tile([128, 11, OW], f16, tag="vn", name="vn")
                        nc.vector.tensor_tensor(
                            vx[:, 0:nh, :],
                            y3c[:, or0 : or0 + nor : 2, :],
                            y3c[:, or0 + 1 : or0 + nor : 2, :],
                            OP.max,
                        )
                        nc.vector.tensor_tensor(
                            vn[:, 0:nh, :],
                            y3c[:, or0 : or0 + nor : 2, :],
                            y3c[:, or0 + 1 : or0 + nor : 2, :],
                            OP.min,
                        )
                        nc.vector.tensor_tensor(
                            hx[:, h0 : h0 + nh, :],
                            vx[:, 0:nh, 0 : OW : 2],
                            vx[:, 0:nh, 1 : OW : 2],
                            OP.max,
                        )
                        nc.vector.tensor_tensor(
                            hn[:, h0 : h0 + nh, :],
                            vn[:, 0:nh, 0 : OW : 2],
                            vn[:, 0:nh, 1 : OW : 2],
                            OP.min,
                        )
                    if pending is not None:
                        if ci == 0:
                            tail_stats(pending)
                        elif ci == 1:
                            tail_gsum(pending)
                        elif pending["pf"]:
                            if ci == 2:
                                tail_combine(pending)
                            elif ci == 3:
                                tail_store(pending)
                                pending = None
                        elif ci == 2:
                            tail_affine(pending)
                        elif ci == 3:
                            tail_vpool(pending)
                        elif ci == 4:
                            tail_hpool(pending)
                        elif ci == 6:
                            tail_store(pending)
                            pending = None

                pending = {"b": b, "s1c": s1cols, "sqc": sqcols, "y": y_raw,
                           "st": stats, "n_act": n_act, "n_bn": n_bn,
                           "n_act_els": n_act_els, "pf": pool_first}
                if pool_first:
                    pending["hx"], pending["hn"] = hx, hn
            tail_stats(pending)
            tail_gsum(pending)
            tail_combine(pending, 0, 32)
            tail_store(pending, 0, 32)
            tail_combine(pending, 32, PH)
            tail_store(pending, 32, PH)
    nc.finalize()
    _CACHED["nc"] = nc
    return nc


def _prep_consts(conv_w, conv_b, gn_w, gn_b, scale):
    # wp[kw, ci + 64*kh, co] = conv_w[co, ci, kh, kw] for kh in {0,1}
    # w2[ci, co] = conv_w[co, ci, 2, 0]; w2[64+ci, co] = conv_w[co, ci, 2, 1]
    # ws[ci, co] = conv_w[co, ci, 2, 2]
    w = np.ascontiguousarray(conv_w.astype(np.float32))
    wp = np.empty((128, 3 * COUT), np.float16)
    w2 = np.empty((128, COUT), np.float16)
    ws = np.empty((128, COUT), np.float16)
    for kw in range(3):
        wp[0:64, kw * COUT : (kw + 1) * COUT] = w[:, :, 0, kw].T
        wp[64:128, kw * COUT : (kw + 1) * COUT] = w[:, :, 1, kw].T
    w2[0:64, :] = w[:, :, 2, 0].T
    w2[64:128, :] = w[:, :, 2, 1].T
    ws[0:64, :] = w[:, :, 2, 2].T
    ws[64:128, :] = w[:, :, 2, 2].T
    cb = conv_b.astype(np.float32).reshape(COUT, 1)
    sc = scale.astype(np.float32).reshape(COUT)
    gs = (gn_w.astype(np.float32) * sc).reshape(COUT, 1)
    gbs = (gn_b.astype(np.float32) * sc).reshape(COUT, 1)
    bones = np.zeros((COUT, COUT), np.float16)
    for g in range(NG):
        bones[g * GSZ : (g + 1) * GSZ, g * GSZ : (g + 1) * GSZ] = 1.0
    return wp, w2, ws, cb, gs, gbs, bones


def kernel(x, conv_w, conv_b, gn_w, gn_b, scale):
    x = np.asarray(x, dtype=np.float32).astype(np.float16)
    wp, w2, ws, cb, gs, gbs, bones = _prep_consts(
        np.asarray(conv_w), np.asarray(conv_b), np.asarray(gn_w),
        np.asarray(gn_b), np.asarray(scale),
    )
    nc = _build()
    in_maps = []
    for c in range(N_CORES):
        in_maps.append({
            "xs": x[c * BPC : (c + 1) * BPC],
            "wp": wp, "w2": w2, "ws": ws,
            "cb": cb, "gs": gs, "gbs": gbs, "bones": bones,
        })
    results = _run_cached(nc, in_maps)
    out = np.concatenate([results[c]["out"] for c in range(N_CORES)], axis=0)
    return out.astype(np.float32)


def _run_cached(nc, in_maps):
    """run_bass_kernel_spmd's axon path with the jitted executable cached
    across calls (avoids re-tracing the shard_map wrapper every call)."""
    import jax
    import numpy as _np
    from jax.sharding import Mesh, PartitionSpec
    from jax.experimental.shard_map import shard_map
    from concourse import bass2jax

    if "runner" not in _CACHED:
        bass2jax.install_neuronx_cc_hook()
        partition_name = (
            nc.partition_id_tensor.name if nc.partition_id_tensor else None
        )
        in_names, out_names, out_avals, zero_outs = [], [], [], []
        for alloc in nc.m.functions[0].allocations:
            if not isinstance(alloc, mybir.MemoryLocationSet):
                continue
            name = alloc.memorylocations[0].name
            if alloc.kind == "ExternalInput":
                if name != partition_name:
                    in_names.append(name)
            elif alloc.kind == "ExternalOutput":
                shape = tuple(alloc.tensor_shape)
                dtype = mybir.dt.np(alloc.dtype)
                out_names.append(name)
                out_avals.append(jax.core.ShapedArray(shape, dtype))
                zero_outs.append(_np.zeros(shape, dtype))
        n_params = len(in_names)
        n_outs = len(out_avals)
        all_names = list(in_names) + list(out_names)
        if partition_name is not None:
            all_names.append(partition_name)
        donate = tuple(range(n_params, n_params + n_outs))

        def _body(*args):
            operands = list(args)
            if partition_name is not None:
                operands.append(bass2jax.partition_id_tensor())
            outs = bass2jax._bass_exec_p.bind(
                *operands,
                out_avals=tuple(out_avals),
                in_names=tuple(all_names),
                out_names=tuple(out_names),
                lowering_input_output_aliases=(),
                sim_require_finite=True,
                sim_require_nnan=True,
                nc=nc,
            )
            return tuple(outs)

        devices = jax.devices()[:N_CORES]
        mesh = Mesh(_np.asarray(devices), ("core",))
        in_specs = (PartitionSpec("core"),) * (n_params + n_outs)
        out_specs = (PartitionSpec("core"),) * n_outs
        sharded = jax.jit(
            shard_map(_body, mesh=mesh, in_specs=in_specs,
                      out_specs=out_specs, check_rep=False),
            donate_argnums=donate, keep_unused=True,
        )
        _CACHED["runner"] = (sharded, in_names, out_names, out_avals, zero_outs)

    sharded, in_names, out_names, out_avals, zero_outs = _CACHED["runner"]
    import numpy as _np2
    concat_in = [
        _np2.concatenate([_np2.asarray(in_maps[c][n]) for c in range(N_CORES)], axis=0)
        for n in in_names
    ]
    concat_zeros = [
        _np2.zeros((N_CORES * z.shape[0], *z.shape[1:]), z.dtype) for z in zero_outs
    ]
    out_arrs = sharded(*concat_in, *concat_zeros)
    return [
        {
            name: _np2.asarray(out_arrs[i]).reshape(N_CORES, *out_avals[i].shape)[c]
            for i, name in enumerate(out_names)
        }
        for c in range(N_CORES)
    ]


if __name__ == "__main__":
    rng = np.random.default_rng(0)
    x = rng.standard_normal((B_FULL, CIN, H, W), dtype=np.float32)
    cw = rng.standard_normal((COUT, CIN, 3, 3), dtype=np.float32)
    out = kernel(x, cw, rng.standard_normal(COUT, dtype=np.float32),
                 rng.standard_normal(COUT, dtype=np.float32),
                 rng.standard_normal(COUT, dtype=np.float32),
                 rng.standard_normal((COUT, 1, 1), dtype=np.float32))
    print(out.shape, out.dtype)


# revision 33
# speedup vs baseline: 1.0995x; 1.0154x over previous
"""Fused conv3x3 -> GroupNorm(16) -> channel scale -> maxpool2x2 -> clamp[0,1]
Trainium2 Bass kernel, data-parallel over batch on 8 NeuronCores.

Input  x [32, 64, 128, 128] f32  -> output [32, 128, 63, 63] f32.
Each core handles 4 samples.

Conv: fp16 tap-pair matmuls, 5 PE passes per 8-row output group half
(vs 9 naive):
  - xt buffer: partition ci holds x[ci, row], ci+64 holds x[ci, row+1]
    -> one [128,128] stacked weight covers taps (kh0,kw)+(kh1,kw): 3 passes
  - xq buffer: both blocks hold x[ci, row+2], block1 shifted 1 column
    (loaded as flat row-major slices at +0/+1 element offsets, so both are
    single contiguous DMAs) -> taps (kh2,0)+(kh2,1) in 1 pass; (kh2,2)
    reads xq block0 at column offset 2 as a 64-contraction pass.
The two 64-contraction passes land on disjoint PE row groups so their
execution overlaps; PE busy ~157us/core is within ~20% of the MAC roofline.

Tail strategy (keeps the PE fed; DVE and ACT each stay under the PE's
~39us/sample):
  - stats WITHOUT bn_stats: the ACT PSUM-evacuation Copy produces per-group
    sum(y) via accum_out; one DVE tensor_tensor_reduce per group (y*y ->
    scratch, accum add) produces sum(y^2). Conv bias is folded analytically.
  - affine BEFORE pooling, as a single in-place DVE tensor_scalar over the
    whole sample -- contiguous fp16 SBUF operands hit the DVE 4x perf mode
    (~0.26 ns/el), so this is 3x cheaper than splitting affine over ACT+DVE.
  - single max-pool branch, vertical pairs first (contiguous innermost ->
    DVE 2x mode), then horizontal pairs (strided, 1x), then fused clamp.
  - per-sample tails (coeff chain / affine / pools / store) are emitted
    interleaved with the next sample's conv chunks so no queue head-blocks.
fp16 output upcast to f32 on host.
"""

import numpy as np

import concourse.bacc as bacc
import concourse.mybir as mybir
import concourse.tile as tile
from concourse.bass_utils import run_bass_kernel_spmd

N_CORES = 8
B_FULL, CIN, H, W = 32, 64, 128, 128
COUT = 128
BPC = B_FULL // N_CORES  # samples per core
OH = OW = 126
PH = PW = 63
NG = 16  # groups
GSZ = COUT // NG  # 8 channels per group
EPS = 1e-5
S = OH * OW  # spatial size per sample
NEL = float(S)  # elements per channel for stats

# (x_row0, n_xrows, out_row0, n_out_rows)
CHUNKS = [(0, 10, 0, 8), (8, 10, 8, 8), (16, 10, 16, 8), (24, 14, 24, 12),
          (36, 18, 36, 16), (52, 24, 52, 22), (74, 24, 74, 22), (96, 24, 96, 22),
          (118, 10, 118, 8)]
XROWS_MAX = 24
NGROUPS = 17  # total 8-or-smaller row groups per sample
# row count per group (CHUNKS split into <=8-row groups)
GROUP_ROWS = [8, 8, 8, 8, 4, 8, 8, 8, 8, 6, 8, 8, 6, 8, 8, 6, 8]
# groups whose stats run on ACT (Square+accum); rest use DVE bn_stats.
# the last sample shifts most groups to ACT: its DVE also carries the
# previous sample's tail plus the last-sample max+min pools, and any DVE
# backlog there directly lengthens the serial post-conv tail
A6 = frozenset({1, 4, 7, 10, 13, 16})
A12 = frozenset(range(NGROUPS)) - {0, 4, 8, 12, 16}
ACT_SETS = [A6, A6, A12, A12]
# samples 2,3 pool during their own conv (max+min branches) and get a short
# all-DVE affine-combine tail; samples 0,1 use the cheaper affine-first tail
POOL_FIRST = (False, False, True, True)
N_ACT_MAX = max(len(s) for s in ACT_SETS)
N_BN_MAX = max(NGROUPS - len(s) for s in ACT_SETS)

_CACHED = {}


def _build():
    if "nc" in _CACHED:
        return _CACHED["nc"]
    f32 = mybir.dt.float32
    f16 = mybir.dt.float16
    AF = mybir.ActivationFunctionType
    OP = mybir.AluOpType

    nc = bacc.Bacc("TRN2", target_bir_lowering=False, debug=False)
    xs = nc.dram_tensor("xs", [BPC, CIN, H, W], f16, kind="ExternalInput").ap()
    wp_d = nc.dram_tensor("wp", [128, 3 * COUT], f16, kind="ExternalInput").ap()
    w2_d = nc.dram_tensor("w2", [128, COUT], f16, kind="ExternalInput").ap()
    ws_d = nc.dram_tensor("ws", [128, COUT], f16, kind="ExternalInput").ap()
    cb_d = nc.dram_tensor("cb", [COUT, 1], f32, kind="ExternalInput").ap()
    gs_d = nc.dram_tensor("gs", [COUT, 1], f32, kind="ExternalInput").ap()
    gbs_d = nc.dram_tensor("gbs", [COUT, 1], f32, kind="ExternalInput").ap()
    bones_d = nc.dram_tensor("bones", [COUT, COUT], f16, kind="ExternalInput").ap()
    out_d = nc.dram_tensor("out", [BPC, COUT, PH, PW], f16, kind="ExternalOutput").ap()

    with tile.TileContext(nc) as tc:
        with (
            tc.tile_pool(name="consts", bufs=1) as cpool,
            tc.tile_pool(name="xpool", bufs=4) as xpool,
            tc.tile_pool(name="xqpool", bufs=4) as xqpool,
            tc.tile_pool(name="ypool", bufs=2) as ypool,
            tc.tile_pool(name="sqpool", bufs=1) as sqpool,
            tc.tile_pool(name="stpool", bufs=2) as stpool,
            tc.tile_pool(name="pvpool", bufs=1) as pvpool,
            tc.tile_pool(name="popool", bufs=2) as popool,
            tc.tile_pool(name="vspool", bufs=1) as vspool,
            tc.tile_pool(name="hxpool", bufs=2) as hxpool,
            tc.tile_pool(name="cps", bufs=3, space="PSUM") as cps,
            tc.tile_pool(name="gps", bufs=1, space="PSUM") as gps,
        ):
            wp = cpool.tile([128, 3 * COUT], f16, name="wp_t")
            w2 = cpool.tile([128, COUT], f16, name="w2_t")
            ws = cpool.tile([128, COUT], f16, name="ws_t")
            cb = cpool.tile([COUT, 1], f32, name="cb_t")
            gs = cpool.tile([COUT, 1], f32, name="gs_t")
            gbs = cpool.tile([COUT, 1], f32, name="gbs_t")
            bones = cpool.tile([COUT, COUT], f16, name="bones_t")
            zeros1 = cpool.tile([COUT, 1], f32, name="zeros1")
            nc.vector.memset(zeros1[:], 0.0)

            def load_chunk(b, xr0, nor):
                xt = xpool.tile([128, XROWS_MAX, W], f16, tag="x", name="xt")
                nc.sync.dma_start(
                    xt[0:64, 0:nor, :], xs[b, :, xr0 : xr0 + nor, :]
                )
                nc.sync.dma_start(
                    xt[64:128, 0:nor, :], xs[b, :, xr0 + 1 : xr0 + 1 + nor, :]
                )
                xq = xqpool.tile([128, XROWS_MAX, W], f16, tag="xq", name="xq")
                xf = xs[b].rearrange("c h w -> c (h w)")
                off = (xr0 + 2) * W
                nc.sync.dma_start(
                    xq[0:64, 0:nor, :].rearrange("p a b -> p (a b)"),
                    xf[:, off : off + nor * W],
                )
                n2 = min(nor * W, H * W - off - 1)
                nc.sync.dma_start(
                    xq[64:128, 0:nor, :].rearrange("p a b -> p (a b)")[:, 0:n2],
                    xf[:, off + 1 : off + 1 + n2],
                )
                return xt, xq

            # first matmul needs wp + chunk-0 x: issue those DMAs first, the
            # remaining consts (not needed until later matmuls / tails) after
            nc.sync.dma_start(wp[:], wp_d[:])
            prefetch = load_chunk(0, CHUNKS[0][0], CHUNKS[0][3])
            nc.sync.dma_start(w2[:], w2_d[:])
            nc.sync.dma_start(ws[:], ws_d[:])
            nc.sync.dma_start(cb[:], cb_d[:])
            nc.sync.dma_start(gs[:], gs_d[:])
            nc.sync.dma_start(gbs[:], gbs_d[:])
            nc.sync.dma_start(bones[:], bones_d[:])
            # scratch for the ACT Square main output (discarded)
            sqscr = sqpool.tile([128, 8 * OW], f16, name="sqscr")

            def tail_stats(tl):
                # merge bn_stats Welford aggregate (N_BN_ELS els) with the
                # ACT-square raw sums (N_ACT_ELS els) into full-sample
                # st0 = E[y+cb], st1 = E[(y+cb)^2]
                n_bn_els = NEL - tl["n_act_els"]
                mv = stpool.tile([128, 2], f32, tag="mv", name="mv")
                nc.vector.bn_aggr(mv[:], tl["st"][:, 0 : 2 * tl["n_bn"], :])
                S1 = stpool.tile([128, 1], f32, tag="S1", name="S1")
                nc.vector.tensor_reduce(S1[:], tl["s1c"][:, 0 : tl["n_act"]],
                                        mybir.AxisListType.XYZW, OP.add)
                S2 = stpool.tile([128, 1], f32, tag="S2", name="S2")
                nc.vector.tensor_reduce(S2[:], tl["sqc"][:, 0 : tl["n_act"]],
                                        mybir.AxisListType.XYZW, OP.add)
                # S1 <- S1 + mean_a * N_a ; S2 <- S2 + (var_a+mean_a^2) * N_a
                t0 = stpool.tile([128, 1], f32, tag="t0", name="t0")
                nc.vector.tensor_scalar(t0[:], mv[:, 0:1], n_bn_els, S1[:],
                                        OP.mult, OP.add)
                t2 = stpool.tile([128, 1], f32, tag="t2", name="t2")
                nc.vector.tensor_tensor(t2[:], mv[:, 0:1], mv[:, 0:1], OP.mult)
                nc.vector.tensor_tensor(t2[:], t2[:], mv[:, 1:2], OP.add)
                nc.vector.tensor_scalar(t2[:], t2[:], n_bn_els, S2[:],
                                        OP.mult, OP.add)
                st = stpool.tile([128, 2], f32, tag="sts", name="st")
                # st0 = S1/N + cb ; st1 = S2/N + cb*(2*S1/N + cb)
                nc.vector.tensor_scalar(st[:, 0:1], t0[:], 1.0 / NEL, cb[:],
                                        OP.mult, OP.add)
                t1 = stpool.tile([128, 1], f32, tag="t1", name="t1")
                nc.vector.tensor_scalar(t1[:], t0[:], 2.0 / NEL, cb[:],
                                        OP.mult, OP.add)
                nc.vector.tensor_tensor(t1[:], t1[:], cb[:], OP.mult)
                nc.vector.tensor_scalar(st[:, 1:2], t2[:], 1.0 / NEL, None,
                                        OP.mult)
                nc.vector.tensor_tensor(st[:, 1:2], st[:, 1:2], t1[:], OP.add)
                stf = stpool.tile([128, 2], f16, tag="stf", name="stf")
                nc.vector.tensor_scalar(stf[:], st[:], 1.0, None, OP.mult)
                tl["stv"] = stf

            def tail_gsum(tl):
                st = tl["stv"]
                gsum = gps.tile([128, 2], f32, tag="gsum", name="gsum")
                nc.tensor.matmul(gsum[:], bones[:], st[:], start=True, stop=True)
                mgrp = stpool.tile([128, 1], f32, tag="mgrp", name="mgrp")
                nc.vector.tensor_scalar(
                    mgrp[:], gsum[:, 0:1], 1.0 / GSZ, None, OP.mult
                )
                vgrp = stpool.tile([128, 1], f32, tag="vgrp", name="vgrp")
                nc.vector.tensor_scalar(
                    vgrp[:], gsum[:, 1:2], 1.0 / GSZ, EPS, OP.mult, OP.add
                )
                msq = stpool.tile([128, 1], f32, tag="msq", name="msq")
                nc.vector.tensor_tensor(msq[:], mgrp[:], mgrp[:], OP.mult)
                nc.vector.tensor_tensor(vgrp[:], vgrp[:], msq[:], OP.subtract)
                sdev = stpool.tile([128, 1], f32, tag="sdev", name="sdev")
                nc.scalar.activation(sdev[:], vgrp[:], AF.Sqrt, bias=zeros1[:])
                inv = stpool.tile([128, 1], f32, tag="inv", name="inv")
                nc.vector.reciprocal(inv[:], sdev[:])
                Acoef = stpool.tile([128, 1], f32, tag="Ac", name="Acoef")
                nc.vector.tensor_tensor(Acoef[:], inv[:], gs[:], OP.mult)
                Bcoef = stpool.tile([128, 1], f32, tag="Bc", name="Bcoef")
                nc.vector.tensor_tensor(Bcoef[:], cb[:], mgrp[:], OP.subtract)
                nc.vector.tensor_tensor(Bcoef[:], Bcoef[:], Acoef[:], OP.mult)
                nc.vector.tensor_tensor(Bcoef[:], Bcoef[:], gbs[:], OP.add)
                tl["A"], tl["B"] = Acoef, Bcoef

            def tail_affine(tl):
                # z = A*y + B in place, whole sample (DVE 4x mode)
                y = tl["y"]
                nc.vector.tensor_scalar(
                    y[:], y[:], tl["A"][:], tl["B"][:], OP.mult, OP.add
                )

            def tail_vpool(tl):
                # vertical max over row pairs: [126,126] -> [63,126]
                y3 = tl["y"][:].rearrange("p (a b) -> p a b", b=OW)
                pv = pvpool.tile([128, PH, OW], f16, tag="pv", name="pv")
                nc.vector.tensor_tensor(
                    pv[:], y3[:, 0:OH:2, :], y3[:, 1:OH:2, :], OP.max
                )
                tl["pv"] = pv

            def tail_hpool(tl):
                # horizontal max over col pairs + clamp: [63,126] -> [63,63]
                pv = tl["pv"]
                po = popool.tile([128, PH, PW], f16, tag="po", name="po")
                nc.vector.tensor_tensor(
                    po[:], pv[:, :, 0:OW:2], pv[:, :, 1:OW:2], OP.max
                )
                nc.vector.tensor_scalar(po[:], po[:], 1.0, 0.0, OP.min, OP.max)
                tl["po"] = po

            def tail_combine(tl, r0=0, r1=PH):
                # z = clamp(max(A*maxpool(y)+B, A*minpool(y)+B), 0, 1)
                # == clamp(maxpool(A*y+B)) for either sign of A
                Acoef, Bcoef = tl["A"], tl["B"]
                hx_, hn_ = tl["hx"], tl["hn"]
                if r0 == 0:
                    tl["po"] = popool.tile([128, PH, PW], f16, tag="po",
                                           name="po")
                po = tl["po"]
                nc.vector.tensor_scalar(
                    po[:, r0:r1, :], hx_[:, r0:r1, :], Acoef[:], Bcoef[:],
                    OP.mult, OP.add
                )
                nc.vector.tensor_scalar(
                    hn_[:, r0:r1, :], hn_[:, r0:r1, :], Acoef[:], Bcoef[:],
                    OP.mult, OP.add
                )
                nc.vector.tensor_tensor(po[:, r0:r1, :], po[:, r0:r1, :],
                                        hn_[:, r0:r1, :], OP.max)
                nc.vector.tensor_scalar(po[:, r0:r1, :], po[:, r0:r1, :],
                                        1.0, 0.0, OP.min, OP.max)

            def tail_store(tl, r0=0, r1=PH):
                # dispatch from the idle GpSimd SWDGE: the store waits on the
                # DVE clamp, and on either hardware DGE queue (Sync carries
                # x-loads, ACT carries PSUM evacs) it would head-block work
                # the PE depends on, starving it for ~5-10us per sample
                nc.gpsimd.dma_start(
                    out_d[tl["b"], :, r0:r1, :].rearrange("c h w -> c (h w)"),
                    tl["po"][:, r0:r1, :].rearrange("p a b -> p (a b)"),
                )

            pending = None
            for b in range(BPC):
                pool_first = POOL_FIRST[b]
                if pool_first:
                    hx = hxpool.tile([128, PH, PW], f16, tag="hx", name="hx")
                    hn = hxpool.tile([128, PH, PW], f16, tag="hn", name="hn")
                y_raw = ypool.tile([128, S], f16, tag="y", name="y_raw")
                act_set = ACT_SETS[b]
                n_act = len(act_set)
                n_bn = NGROUPS - n_act
                n_act_els = float(sum(GROUP_ROWS[g] for g in act_set) * OW)
                s1cols = stpool.tile([128, N_ACT_MAX], f32, tag="s1c", name="s1cols")
                sqcols = stpool.tile([128, N_ACT_MAX], f32, tag="sqc", name="sqcols")
                stats = stpool.tile([128, 2 * N_BN_MAX, 6], f32,
                                    tag="st", name="stats")

                gi = 0  # group index within sample
                aci = 0  # ACT-square group counter
                si = 0  # bn_stats slot counter
                for ci, (xr0, nxr, or0, nor) in enumerate(CHUNKS):
                    # xt block0 = x rows xr0.., block1 = x rows xr0+1..;
                    # only `nor` rows each are read (kh2 taps come from xq).
                    # xq holds x rows (xr0+2) duplicated with a 1-column shift
                    # between partition blocks -> covers taps (kh2,kw0)+(kh2,kw1)
                    # in one 128-contraction matmul; block0 also serves (kh2,kw2)
                    if b == 0 and ci == 0:
                        xt, xq = prefetch
                    else:
                        xt, xq = load_chunk(b, xr0, nor)

                    g0 = or0
                    while g0 < or0 + nor:
                        gn = min(8, or0 + nor - g0)  # 8, 6 or 4 output rows
                        hr = gn // 2  # rows per half
                        cp = cps.tile([128, 1024], f32, tag="cp", name="cp")
                        for half in range(2):
                            row0 = g0 + half * hr
                            l0 = row0 - xr0
                            outap = cp[:, half * 512 : half * 512 + hr * OW]
                            for kw in range(3):
                                nc.tensor.matmul(
                                    outap,
                                    wp[:, kw * COUT : (kw + 1) * COUT],
                                    xt[:, l0 : l0 + hr, kw : kw + OW],
                                    start=(kw == 0),
                                    stop=False,
                                )
                            nc.tensor.matmul(
                                outap,
                                w2[:],
                                xq[:, l0 : l0 + hr, 0:OW],
                                start=False,
                                stop=False,
                            )
                        # (kh2,kw2) singles for both halves, adjacent on
                        # disjoint PE row groups (0-63 / 64-127) so the
                        # 16x 32x32 sub-arrays overlap their execution.
                        # half1 reads xq block1 (data shifted +1 col) at
                        # offset 1 -> x column c+2, same tap.
                        l0a = g0 - xr0
                        l0b = g0 + hr - xr0
                        nc.tensor.matmul(
                            cp[:, 0 : hr * OW],
                            ws[0:64, :],
                            xq[0:64, l0a : l0a + hr, 2 : 2 + OW],
                            start=False,
                            stop=True,
                            skip_group_check=True,
                        )
                        nc.tensor.matmul(
                            cp[:, 512 : 512 + hr * OW],
                            ws[64:128, :],
                            xq[64:128, l0b : l0b + hr, 1 : 1 + OW],
                            start=False,
                            stop=True,
                            skip_group_check=True,
                        )
                        # evacuate both halves in one strided ACT copy;
                        # accum_out gives this group's per-channel sum(y)
                        yv = y_raw[:, g0 * OW : (g0 + gn) * OW].rearrange(
                            "p (a b) -> p a b", b=hr * OW
                        )
                        on_act = gi in act_set
                        nc.scalar.activation(
                            yv,
                            cp[:].rearrange("p (a b) -> p a b", b=512)[
                                :, :, 0 : hr * OW
                            ],
                            AF.Copy,
                            accum_out=(s1cols[:, aci : aci + 1]
                                       if on_act else None),
                        )
                        yseg = y_raw[:, g0 * OW : (g0 + gn) * OW]
                        if on_act:
                            # ACT square pass: accum gives sum(y^2)
                            nc.scalar.activation(
                                sqscr[:, 0 : gn * OW], yseg, AF.Square,
                                accum_out=sqcols[:, aci : aci + 1],
                            )
                            aci += 1
                        else:
                            # DVE one-pass Welford stats per 504-el half
                            for half in range(2):
                                r0 = (g0 + half * hr) * OW
                                nc.vector.bn_stats(
                                    stats[:, si, :],
                                    y_raw[:, r0 : r0 + hr * OW],
                                )
                                si += 1
                        gi += 1
                        g0 += gn

                    if pool_first:
                        # pool during the sample's own conv (max+min branches
                        # on raw y) so only a short affine-combine tail is left
                        y3c = y_raw[:].rearrange("p (a b) -> p a b", b=OW)
                        h0 = or0 // 2
                        nh = nor // 2
                        vx = vspool.tile([128, 11, OW], f16, tag="vx", name="vx")
                        vn = vspool.tile([128, 11, OW], f16, tag="vn", name="vn")
                        nc.vector.tensor_tensor(
                            vx[:, 0:nh, :],
                            y3c[:, or0 : or0 + nor : 2, :],
                            y3c[:, or0 + 1 : or0 + nor : 2, :],
                            OP.max,
                        )
                        nc.vector.tensor_tensor(
                            vn[:, 0:nh, :],
                            y3c[:, or0 : or0 + nor : 2, :],
                            y3c[:, or0 + 1 : or0 + nor : 2, :],
                            OP.min,
                        )
                        nc.vector.tensor_tensor(
                            hx[:, h0 : h0 + nh, :],
                            vx[:, 0:nh, 0 : OW : 2],
                            vx[:, 0:nh, 1 : OW : 2],
                            OP.max,
                        )
                        nc.vector.tensor_tensor(
                            hn[:, h0 : h0 + nh, :],
                            vn[:, 0:nh, 0 : OW : 2],
                            vn[:, 0:nh, 1 : OW : 2],
                            OP.min,
                        )
                    if pending is not None:
                        if ci == 0:
                            tail_stats(pending)
                        elif ci == 1:
                            tail_gsum(pending)
                        elif pending["pf"]:
                            if ci == 2:
                                tail_combine(pending)
                            elif ci == 3:
                                tail_store(pending)
                                pending = None
                        elif ci == 2:
                            tail_affine(pending)
                        elif ci == 3:
                            tail_vpool(pending)
                        elif ci == 4:
                            tail_hpool(pending)
                        elif ci == 6:
                            tail_store(pending)
                            pending = None

                pending = {"b": b, "s1c": s1cols, "sqc": sqcols, "y": y_raw,
                           "st": stats, "n_act": n_act, "n_bn": n_bn,
                           "n_act_els": n_act_els, "pf": pool_first}
                if pool_first:
                    pending["hx"], pending["hn"] = hx, hn
            tail_stats(pending)
            tail_gsum(pending)
            tail_combine(pending, 0, 32)
            tail_store(pending, 0, 32)
            tail_combine(pending, 32, PH)
            tail_store(pending, 32, PH)
    nc.finalize()
    _CACHED["nc"] = nc
    return nc


def _prep_consts(conv_w, conv_b, gn_w, gn_b, scale):
    # wp[kw, ci + 64*kh, co] = conv_w[co, ci, kh, kw] for kh in {0,1}
    # w2[ci, co] = conv_w[co, ci, 2, 0]; w2[64+ci, co] = conv_w[co, ci, 2, 1]
    # ws[ci, co] = conv_w[co, ci, 2, 2]
    w = np.ascontiguousarray(conv_w.astype(np.float32))
    wp = np.empty((128, 3 * COUT), np.float16)
    w2 = np.empty((128, COUT), np.float16)
    ws = np.empty((128, COUT), np.float16)
    for kw in range(3):
        wp[0:64, kw * COUT : (kw + 1) * COUT] = w[:, :, 0, kw].T
        wp[64:128, kw * COUT : (kw + 1) * COUT] = w[:, :, 1, kw].T
    w2[0:64, :] = w[:, :, 2, 0].T
    w2[64:128, :] = w[:, :, 2, 1].T
    ws[0:64, :] = w[:, :, 2, 2].T
    ws[64:128, :] = w[:, :, 2, 2].T
    cb = conv_b.astype(np.float32).reshape(COUT, 1)
    sc = scale.astype(np.float32).reshape(COUT)
    gs = (gn_w.astype(np.float32) * sc).reshape(COUT, 1)
    gbs = (gn_b.astype(np.float32) * sc).reshape(COUT, 1)
    bones = np.zeros((COUT, COUT), np.float16)
    for g in range(NG):
        bones[g * GSZ : (g + 1) * GSZ, g * GSZ : (g + 1) * GSZ] = 1.0
    return wp, w2, ws, cb, gs, gbs, bones


def kernel(x, conv_w, conv_b, gn_w, gn_b, scale):
    x = np.asarray(x, dtype=np.float32).astype(np.float16)
    wp, w2, ws, cb, gs, gbs, bones = _prep_consts(
        np.asarray(conv_w), np.asarray(conv_b), np.asarray(gn_w),
        np.asarray(gn_b), np.asarray(scale),
    )
    nc = _build()
    in_maps = []
    for c in range(N_CORES):
        in_maps.append({
            "xs": x[c * BPC : (c + 1) * BPC],
            "wp": wp, "w2": w2, "ws": ws,
            "cb": cb, "gs": gs, "gbs": gbs, "bones": bones,
        })
    results = _run_cached(nc, in_maps)
    out = np.concatenate([results[c]["out"] for c in range(N_CORES)], axis=0)
    return out.astype(np.float32)


def _run_cached(nc, in_maps):
    """run_bass_kernel_spmd's axon path with the jitted executable cached
    across calls (avoids re-tracing the shard_map wrapper every call)."""
    import jax
    import numpy as _np
    from jax.sharding import Mesh, PartitionSpec
    from jax.experimental.shard_map import shard_map
    from concourse import bass2jax

    if "runner" not in _CACHED:
        bass2jax.install_neuronx_cc_hook()
        partition_name = (
            nc.partition_id_tensor.name if nc.partition_id_tensor else None
        )
        in_names, out_names, out_avals, zero_outs = [], [], [], []
        for alloc in nc.m.functions[0].allocations:
            if not isinstance(alloc, mybir.MemoryLocationSet):
                continue
            name = alloc.memorylocations[0].name
            if alloc.kind == "ExternalInput":
                if name != partition_name:
                    in_names.append(name)
            elif alloc.kind == "ExternalOutput":
                shape = tuple(alloc.tensor_shape)
                dtype = mybir.dt.np(alloc.dtype)
                out_names.append(name)
                out_avals.append(jax.core.ShapedArray(shape, dtype))
                zero_outs.append(_np.zeros(shape, dtype))
        n_params = len(in_names)
        n_outs = len(out_avals)
        all_names = list(in_names) + list(out_names)
        if partition_name is not None:
            all_names.append(partition_name)
        donate = tuple(range(n_params, n_params + n_outs))

        def _body(*args):
            operands = list(args)
            if partition_name is not None:
                operands.append(bass2jax.partition_id_tensor())
            outs = bass2jax._bass_exec_p.bind(
                *operands,
                out_avals=tuple(out_avals),
                in_names=tuple(all_names),
                out_names=tuple(out_names),
                lowering_input_output_aliases=(),
                sim_require_finite=True,
                sim_require_nnan=True,
                nc=nc,
            )
            return tuple(outs)

        devices = jax.devices()[:N_CORES]
        mesh = Mesh(_np.asarray(devices), ("core",))
        in_specs = (PartitionSpec("core"),) * (n_params + n_outs)
        out_specs = (PartitionSpec("core"),) * n_outs
        sharded = jax.jit(
            shard_map(_body, mesh=mesh, in_specs=in_specs,
                      out_specs=out_specs, check_rep=False),
            donate_argnums=donate, keep_unused=True,
        )
        _CACHED["runner"] = (sharded, in_names, out_names, out_avals, zero_outs)

    sharded, in_names, out_names, out_avals, zero_outs = _CACHED["runner"]
    import numpy as _np2
    concat_in = [
        _np2.concatenate([_np2.asarray(in_maps[c][n]) for c in range(N_CORES)], axis=0)
        for n in in_names
    ]
    concat_zeros = [
        _np2.zeros((N_CORES * z.shape[0], *z.shape[1:]), z.dtype) for z in zero_outs
    ]
    out_arrs = sharded(*concat_in, *concat_zeros)
    return [
        {
            name: _np2.asarray(out_arrs[i]).reshape(N_CORES, *out_avals[i].shape)[c]
            for i, name in enumerate(out_names)
        }
        for c in range(N_CORES)
    ]


if __name__ == "__main__":
    rng = np.random.default_rng(0)
    x = rng.standard_normal((B_FULL, CIN, H, W), dtype=np.float32)
    cw = rng.standard_normal((COUT, CIN, 3, 3), dtype=np.float32)
    out = kernel(x, cw, rng.standard_normal(COUT, dtype=np.float32),
                 rng.standard_normal(COUT, dtype=np.float32),
                 rng.standard_normal(COUT, dtype=np.float32),
                 rng.standard_normal((COUT, 1, 1), dtype=np.float32))
    print(out.shape, out.dtype)
